# revision 1
# baseline (speedup 1.0000x reference)
"""Trainium2 Bass kernel v4: 2-layer GCN + global mean pool + MLP head.

v1's collective structure (ONE whole-table AllGather per layer, core-major
banking — minimal collective count, since each collective carries ~250us
fixed overhead on this NRT path) combined with v2's compute improvements:
 - dinv = 1/sqrt(1+in-degree) computed on HOST: no on-device degree pass
   (removes ~1/3 of one-hot builds + matmuls + a serial pipeline stage).
 - hs kept SBUF-resident (no hslf DRAM round trip for the self-loop term).
 - relu+bias fused on the scalar (ACT) engine post-transpose.
 - per-graph inverse counts computed on HOST: single [P,DIM] pooling
   matmul (also avoids the PSUM zero-region conflict of a counts column).
"""

import numpy as np

P = 128
DIM = 64


class CFG:
    def __init__(self, n=100000, e=1600000, g=512, cores=8, maxch_call=10):
        self.N = n
        self.E = e
        self.G = g
        self.R = cores
        self.PC = -(-n // cores)
        self.PC = -(-self.PC // P) * P           # 12544
        self.T = self.PC // P                    # 98
        self.NP = self.PC * cores                # 100352
        self.NROWS = self.NP // 2                # pair-packed table rows
        self.BANK = 32768
        self.NB = -(-self.NROWS // self.BANK)    # 2
        self.MAXCH = maxch_call
        self.nqueues = 4
        self.msg_bufs = 32
        self.skip_gather = False
        self.skip_oh = False
        self.skip_ag = False
        self.skip_tail = False
        self.skip_mm = False
        self.repeat = 1


FULL = CFG()


# ---------------------------------------------------- host preprocessing ---

def _prep(edge_index, batch, cfg):
    """v1-style bucketing: tile-groups, per-(bank,group) gather calls, plus
    host-computed dinv and per-graph inverse counts."""
    c = cfg
    src = np.asarray(edge_index[0], dtype=np.int64)
    dst = np.asarray(edge_index[1], dtype=np.int64)
    batch = np.asarray(batch, dtype=np.int64)

    core = dst // c.PC
    tloc = (dst % c.PC) // P
    slot = dst % P
    row2 = src // 2                              # pair-packed table row
    bank = row2 // c.BANK
    par = (src % 2).astype(np.int64)
    ib = (row2 % c.BANK).astype(np.int64)

    key = (((core * c.T + tloc) * c.NB + bank) * 2 + par)
    order = np.lexsort((ib, key))
    key_s = key[order]
    ib_s = ib[order].astype(np.int16)
    slot_s = slot[order].astype(np.float32)

    nkey = c.R * c.T * c.NB * 2
    cnts = np.bincount(key_s, minlength=nkey)
    counts = cnts.reshape(c.R, c.T, c.NB, 2)
    starts_flat = np.concatenate([[0], np.cumsum(cnts)])

    C_tbp = -(-counts.max(axis=0) // P)         # [T, NB, 2]
    for t in range(c.T):
        if C_tbp[t].sum() == 0:
            C_tbp[t, 0, 0] = 1

    groups = []
    t0 = 0
    while t0 < c.T:
        t1 = t0
        while t1 < c.T:
            nch = C_tbp[t0:t1 + 1].sum(axis=(0, 2)).max()
            if nch > c.MAXCH and t1 > t0:
                break
            t1 += 1
        groups.append((t0, t1))
        t0 = t1

    chunk_of = np.zeros((c.T, c.NB, 2), dtype=np.int64)
    calls = []       # (bank, t0, t1, chunk0, nch, idx_col0)
    CH = 0
    icol = 0
    for (t0, t1) in groups:
        for b in range(c.NB):
            ch0 = CH
            for t in range(t0, t1):
                for pp in range(2):
                    chunk_of[t, b, pp] = CH
                    CH += int(C_tbp[t, b, pp])
            nch = CH - ch0
            if nch:
                calls.append((b, t0, t1, ch0, int(nch), icol))
                icol += nch * P // 16
    sched = dict(C_tbp=C_tbp, chunk_of=chunk_of, groups=groups, calls=calls,
                 CH=int(CH), ICOLS=int(icol))

    # host degree -> dinv (self-loop included)
    deg = np.bincount(dst, minlength=c.N).astype(np.float32) + 1.0
    dinv_all = (1.0 / np.sqrt(deg)).astype(np.float32)

    # host per-graph inverse counts for mean pooling
    gcnt = np.maximum(np.bincount(batch, minlength=c.G), 1).astype(np.float32)
    NGT = -(-c.G // P)
    gpad = np.ones(NGT * P, np.float32)
    gpad[:c.G] = gcnt
    gciF = (1.0 / gpad).reshape(NGT, P).T.copy()          # [128, NGT]

    per_core = []
    for r in range(c.R):
        idxw = np.zeros((P, icol), dtype=np.int16)
        dstl = np.full((P, CH), -1.0, dtype=np.float32)
        for (b, t0, t1, ch0, nch, col0) in calls:
            li = np.zeros(nch * P, dtype=np.int16)
            for t in range(t0, t1):
                for pp in range(2):
                    k = ((r * c.T + t) * c.NB + b) * 2 + pp
                    s0, s1 = starts_flat[k], starts_flat[k + 1]
                    n = int(s1 - s0)
                    if n == 0:
                        continue
                    o = int(chunk_of[t, b, pp] - ch0) * P
                    li[o:o + n] = ib_s[s0:s1]
                    cpos = int(chunk_of[t, b, pp])
                    ii = np.arange(n)
                    dstl[ii % P, cpos + ii // P] = slot_s[s0:s1]
            w = li.reshape(-1, 16).T                      # [16, ncol]
            idxw[:, col0:col0 + nch * P // 16] = np.tile(w, (8, 1))
        n0 = r * c.PC
        nreal = max(0, min(c.N - n0, c.PC))
        bat = np.full(c.PC, -1.0, dtype=np.float32)
        dvi = np.ones(c.PC, dtype=np.float32)
        if nreal > 0:
            bat[:nreal] = batch[n0:n0 + nreal].astype(np.float32)
            dvi[:nreal] = dinv_all[n0:n0 + nreal]
            g0 = int(batch[n0])
            ghi = int(batch[min(n0 + nreal, c.N) - 1])
            assert ghi - g0 < P, (r, g0, ghi)
        else:
            g0 = c.G - 1
        batchF = bat.reshape(c.T, P).T.copy()             # [128, T]
        dinvF = dvi.reshape(c.T, P).T.copy()              # [128, T]
        per_core.append(dict(idxw=idxw, dstl=dstl, batchF=batchF,
                             dinvF=dinvF, g0=g0, gciF=gciF))
    return sched, per_core


# ------------------------------------------------------- program builder ---

def build_program(cfg, sched):
    import concourse.bass as bass
    import concourse.bacc as bacc
    import concourse.mybir as mybir
    import concourse.tile as tile
    from concourse.tile import add_dep_helper

    c = cfg
    dt = mybir.dt
    f32 = dt.float32
    bf16 = dt.bfloat16
    ROWE = 2 * DIM                           # 256B pair-packed bf16 rows
    C_tbp, chunk_of, calls = sched["C_tbp"], sched["chunk_of"], sched["calls"]
    CH, ICOLS = sched["CH"], sched["ICOLS"]
    T, NB = c.T, c.NB

    def tile_chunks(t):
        return [(b, pp, k) for b in range(NB) for pp in range(2)
                for k in range(int(C_tbp[t, b, pp]))]

    def call_chunk0(t, b):
        for (bb, tt0, tt1, c0, nn, _c) in calls:
            if bb == b and tt0 <= t < tt1:
                return c0
        raise AssertionError((t, b))

    nc = bacc.Bacc("TRN2", target_bir_lowering=False, debug=False,
                   num_devices=c.R, num_swdge_queues=c.nqueues)

    # ---- I/O ----
    xT_p = nc.declare_dram_parameter("xT", [DIM, c.PC], f32, isOutput=False)
    W_p = [nc.declare_dram_parameter(f"W{i+1}", [DIM, DIM if i < 3 else 1],
                                     f32, isOutput=False) for i in range(4)]
    b_p = [nc.declare_dram_parameter(f"b{i+1}", [1, DIM if i < 3 else 1],
                                     f32, isOutput=False) for i in range(4)]
    iotaM_p = nc.declare_dram_parameter("iotaM", [P, P], f32, isOutput=False)
    id128_p = nc.declare_dram_parameter("id128", [P, P], f32, isOutput=False)
    ones1_p = nc.declare_dram_parameter("ones1", [1, P], f32, isOutput=False)
    idxw_p = nc.declare_dram_parameter("idxw", [P, ICOLS], dt.int16,
                                       isOutput=False)
    dstl_p = nc.declare_dram_parameter("dstl", [P, CH], f32, isOutput=False)
    batchF_p = nc.declare_dram_parameter("batchF", [P, T], f32, isOutput=False)
    dinvF_p = nc.declare_dram_parameter("dinvF", [P, T], f32, isOutput=False)
    gciF_p = nc.declare_dram_parameter("gciF", [P, -(-c.G // P)], f32,
                                       isOutput=False)
    g0f_p = nc.declare_dram_parameter("g0f", [1, 1], f32, isOutput=False)
    g0i_p = nc.declare_dram_parameter("g0i", [1, 1], dt.uint32, isOutput=False)
    pred_p = nc.declare_dram_parameter("pred", [c.G, 1], f32, isOutput=True)

    # ---- internal DRAM (fresh per repeat for clean timing isolation) ----
    GTOT = -(-(c.G + P) // P) * P
    hsl_a, tabs_a, pool_loc_a, pool_red_a = [], [], [], []
    for r_ in range(c.repeat):
        hsl_a.append([nc.dram_tensor(f"hs{i}_local_{r_}", [c.PC, DIM], bf16)
                      for i in (1, 2)])
        tabs_a.append([nc.dram_tensor(f"hs{i}_table_{r_}", [c.NROWS, ROWE],
                                      bf16)
                       for i in (1, 2)])
        pool_loc_a.append(nc.dram_tensor(f"pool_local_{r_}",
                                         [GTOT, DIM], f32))
        pool_red_a.append(nc.dram_tensor(f"pool_red_{r_}", [GTOT, DIM],
                                         f32, addr_space="Shared"))

    rg = [list(range(c.R))]

    with tile.TileContext(nc) as tc:
        with (
            tc.tile_pool(name="const", bufs=1) as cpool,
            tc.tile_pool(name="msg", bufs=c.msg_bufs) as mpool,
            tc.tile_pool(name="oh", bufs=6) as ohpool,
            tc.tile_pool(name="work", bufs=3) as wpool,
            tc.tile_pool(name="hs1s", bufs=1) as hs1pool,
            tc.tile_pool(name="hs2s", bufs=1) as hs2pool,
            tc.tile_pool(name="ps_agg", bufs=3, space="PSUM") as pagg,
            tc.tile_pool(name="ps_sm", bufs=3, space="PSUM") as psm,
            tc.tile_pool(name="ps_pool", bufs=1, space="PSUM") as ppool,
        ):

            def load(pool, ap, shape, dtype=f32, name=None):
                t = pool.tile(shape, dtype, tag=name)
                nc.sync.dma_start(out=t[:], in_=ap)
                return t

            iotaM = load(cpool, iotaM_p[:, :], [P, P], name="iotaM")
            id128 = load(cpool, id128_p[:, :], [P, P], name="id128")
            ones1 = load(cpool, ones1_p[:, :], [1, P], name="ones1")
            Ws = [load(cpool, W_p[i][:, :], [DIM, DIM if i < 3 else 1],
                       name=f"W{i}") for i in range(4)]
            bs = [load(cpool, b_p[i][:, :], [1, DIM if i < 3 else 1],
                       name=f"b{i}") for i in range(4)]
            idxw = load(cpool, idxw_p[:, :], [P, ICOLS], dt.int16, name="idxw")
            dstl = load(cpool, dstl_p[:, :], [P, CH], name="dstl")
            batchF = load(cpool, batchF_p[:, :], [P, T], name="batchF")
            dinvF = load(cpool, dinvF_p[:, :], [P, T], name="dinvF")
            gciF = load(cpool, gciF_p[:, :], [P, -(-c.G // P)], name="gciF")
            g0f = load(cpool, g0f_p[:, :], [1, 1], name="g0f")

            iotaE = cpool.tile([P, P], bf16, tag="iotaE")
            nc.vector.tensor_copy(out=iotaE[:], in_=iotaM[:])

            # bias broadcast mats via PE outer product: ones1.T @ b
            bB = []
            for i in range(4):
                w = DIM if i < 3 else 1
                pb = psm.tile([P, w], f32, tag="sm")
                nc.tensor.matmul(out=pb[:], lhsT=ones1[:], rhs=bs[i][:],
                                 start=True, stop=True)
                sb = cpool.tile([P, w], f32, tag=f"bB{i}")
                nc.vector.tensor_copy(out=sb[:], in_=pb[:])
                bB.append(sb)
            # b1 as a column [DIM,1] for the ACT fused bias
            pb1 = psm.tile([DIM, 1], f32, tag="sm")
            nc.tensor.transpose(out=pb1[:], in_=bs[0][:],
                                identity=id128[0:1, 0:1])
            b1T = cpool.tile([DIM, 1], f32, tag="b1T")
            nc.vector.tensor_copy(out=b1T[:], in_=pb1[:])
            pg = psm.tile([P, 1], f32, tag="sm")
            nc.tensor.matmul(out=pg[:], lhsT=ones1[:], rhs=g0f[:],
                             start=True, stop=True)
            g0B = cpool.tile([P, 1], f32, tag="g0B")
            nc.vector.tensor_copy(out=g0B[:], in_=pg[:])

            oh_const = cpool.tile([P, P], bf16, tag="ohc")
            nc.vector.memset(oh_const[:], 0.0)

            def onehot_for(chunk):
                if c.skip_oh:
                    return oh_const
                oh = ohpool.tile([P, P], bf16, tag="oh")
                nc.vector.tensor_scalar(
                    out=oh[:], in0=iotaE[:],
                    scalar1=dstl[:, chunk:chunk + 1], scalar2=None,
                    op0=mybir.AluOpType.is_equal)
                return oh

            for _rep in range(c.repeat):
                hsl = hsl_a[_rep]
                tabs = tabs_a[_rep]
                pool_loc = pool_loc_a[_rep]
                pool_red = pool_red_a[_rep]

                hsS = [hs1pool.tile([P, T * DIM], bf16, tag="hs1S",
                                    name="hs1S"),
                       hs2pool.tile([P, T * DIM], bf16, tag="hs2S",
                                    name="hs2S")]
                ags = [None, None]
                hs_dmas = [[], []]

                def emit_hs(i, t, hp):
                    """hs = hp * dinv[t] -> bf16 SBUF resident == AG input."""
                    nc.vector.tensor_scalar(
                        out=hsS[i][:, t * DIM:(t + 1) * DIM], in0=hp[:],
                        scalar1=dinvF[:, t:t + 1], scalar2=None,
                        op0=mybir.AluOpType.mult)
                    d = nc.sync.dma_start(
                        out=hsl[i][t * P:(t + 1) * P, :],
                        in_=hsS[i][:, t * DIM:(t + 1) * DIM])
                    hs_dmas[i].append(d)
                    if t == T - 1 and not c.skip_ag:
                        ag = nc.gpsimd.collective_compute(
                            "AllGather", mybir.AluOpType.bypass,
                            replica_groups=rg,
                            ins=[hsl[i][:, :]], outs=[tabs[i][:, :]])
                        for dd in hs_dmas[i]:
                            add_dep_helper(ag.ins, dd.ins)
                        ags[i] = ag

                # ---------- layer-1 features: h1 = x @ W1 ----------
                # loaded in halves: halves the xTp SBUF peak (24.5KB/part),
                # freeing headroom for deeper msg buffering
                TH_ = T // 2
                with tc.tile_pool(name="xTp", bufs=1) as xpool:
                    for hf in range(2):
                        xh = xpool.tile([DIM, TH_ * P], f32, tag="xh",
                                        name="xh")
                        nc.sync.dma_start(
                            out=xh[:],
                            in_=xT_p[:, hf * TH_ * P:(hf + 1) * TH_ * P])
                        for tt in range(TH_):
                            t = hf * TH_ + tt
                            hp = psm.tile([P, DIM], f32, tag="sm")
                            nc.tensor.matmul(
                                out=hp[:], lhsT=xh[:, tt * P:(tt + 1) * P],
                                rhs=Ws[0][:], start=True, stop=True)
                            emit_hs(0, t, hp)

                # ---------- edge-aggregation layer ----------
                def run_layer(i, fin_fn):
                    gat_of = {}
                    for ci, (b, t0, t1, ch0, nch, col0) in enumerate(calls):
                        m = mpool.tile([P, nch, ROWE], bf16, tag="msg")
                        if not c.skip_gather:
                            g = nc.gpsimd.dma_gather(
                                m[:],
                                tabs[i][b * c.BANK:
                                        min((b + 1) * c.BANK, c.NROWS), :],
                                idxw[:, col0:col0 + nch * P // 16],
                                nch * P, nch * P, ROWE,
                                single_packet=(nch * P <= 1024),
                                queue_num=ci % c.nqueues)
                            if ags[i] is not None:
                                add_dep_helper(g.ins, ags[i].ins)
                        else:
                            nc.vector.memset(m[:, 0, :], 0.0)
                        for t in range(t0, t1):
                            gat_of[(t, b)] = m
                    for t in range(T):
                        chunks = tile_chunks(t)
                        if c.skip_mm:
                            chunks = chunks[:1]
                        pt = pagg.tile([P, DIM], f32, tag="agg")
                        for j, (b, pp, k) in enumerate(chunks):
                            oh = onehot_for(int(chunk_of[t, b, pp]) + k)
                            m = gat_of[(t, b)]
                            kk = (int(chunk_of[t, b, pp])
                                  - call_chunk0(t, b) + k)
                            nc.tensor.matmul(
                                out=pt[:], lhsT=oh[:],
                                rhs=m[:, kk, pp * DIM:(pp + 1) * DIM],
                                start=(j == 0), stop=(j == len(chunks) - 1))
                        fin_fn(t, pt)

                # layer-1 tile epilogue: z -> transpose -> relu+bias -> h2
                def l1fin(t, pt):
                    zt = wpool.tile([P, DIM], f32, tag="zt")
                    nc.vector.tensor_tensor(
                        out=zt[:], in0=pt[:],
                        in1=hsS[0][:, t * DIM:(t + 1) * DIM],
                        op=mybir.AluOpType.add)
                    nc.vector.tensor_scalar(
                        out=zt[:], in0=zt[:], scalar1=dinvF[:, t:t + 1],
                        scalar2=None, op0=mybir.AluOpType.mult)
                    tp = psm.tile([DIM, P], f32, tag="sm")
                    nc.tensor.transpose(out=tp[:], in_=zt[:],
                                        identity=id128[:])
                    o1T = wpool.tile([DIM, P], f32, tag="o1T")
                    nc.scalar.activation(
                        out=o1T[:], in_=tp[:],
                        func=mybir.ActivationFunctionType.Relu,
                        bias=b1T[:, 0:1], scale=1.0)
                    hp2 = psm.tile([P, DIM], f32, tag="sm")
                    nc.tensor.matmul(out=hp2[:], lhsT=o1T[:], rhs=Ws[1][:],
                                     start=True, stop=True)
                    emit_hs(1, t, hp2)

                run_layer(0, l1fin)

                psum_pool = ppool.tile([P, DIM], f32, tag="pool")

                def l2fin(t, pt):
                    zt = wpool.tile([P, DIM], f32, tag="zt2")
                    nc.vector.tensor_tensor(
                        out=zt[:], in0=pt[:],
                        in1=hsS[1][:, t * DIM:(t + 1) * DIM],
                        op=mybir.AluOpType.add)
                    nc.vector.tensor_scalar(
                        out=zt[:], in0=zt[:], scalar1=dinvF[:, t:t + 1],
                        scalar2=None, op0=mybir.AluOpType.mult)
                    og = ohpool.tile([P, P], f32, tag="ohg")
                    nc.vector.tensor_scalar(
                        out=og[:], in0=iotaM[:], scalar1=g0B[:, 0:1],
                        scalar2=batchF[:, t:t + 1], op0=mybir.AluOpType.add,
                        op1=mybir.AluOpType.is_equal)
                    nc.tensor.matmul(out=psum_pool[:], lhsT=og[:],
                                     rhs=zt[:], start=(t == 0),
                                     stop=(t == T - 1))

                run_layer(1, l2fin)

                # ---------- pool finalize + AllReduce + head ----------
                poolsb = wpool.tile([P, DIM], f32, tag="poolsb")
                nc.vector.tensor_copy(out=poolsb[:], in_=psum_pool[:])
                if c.skip_tail:
                    nc.sync.dma_start(out=pred_p[0:P, :], in_=poolsb[:, 0:1])
                else:
                    zz0 = wpool.tile([P, DIM], f32, tag="zz0")
                    nc.vector.memset(zz0[:], 0.0)
                    zdmas = []
                    for qz in range(GTOT // P):
                        zdmas.append(nc.sync.dma_start(
                            out=pool_loc[qz * P:(qz + 1) * P, :], in_=zz0[:]))
                    g0reg = nc.sync.alloc_register(f"g0reg{_rep}")
                    nc.sync.reg_load(g0reg, g0i_p[0:1, 0:1])
                    g0val = nc.sync.snap(g0reg, donate=True, min_val=0,
                                         max_val=GTOT - P)
                    wdma = nc.sync.dma_start(
                        out=pool_loc[bass.ds(g0val, P), :], in_=poolsb[:])
                    for zd in zdmas:
                        add_dep_helper(wdma.ins, zd.ins)
                    ar = nc.gpsimd.collective_compute(
                        "AllReduce", mybir.AluOpType.add, replica_groups=rg,
                        ins=[pool_loc[:, :]], outs=[pool_red[:, :]])
                    add_dep_helper(ar.ins, wdma.ins)

                    for qh in range(-(-c.G // P)):
                        gq = min(P, c.G - qh * P)
                        S = wpool.tile([P, DIM], f32, tag="S")
                        d = nc.sync.dma_start(
                            out=S[:], in_=pool_red[qh * P:qh * P + P, :])
                        add_dep_helper(d.ins, ar.ins)
                        gt = wpool.tile([P, DIM], f32, tag="gt")
                        nc.vector.tensor_scalar(
                            out=gt[:], in0=S[:, 0:DIM],
                            scalar1=gciF[:, qh:qh + 1],
                            scalar2=None, op0=mybir.AluOpType.mult)
                        nc.vector.tensor_tensor(out=gt[:], in0=gt[:],
                                                in1=bB[1][:],
                                                op=mybir.AluOpType.add)
                        tp = psm.tile([DIM, P], f32, tag="sm")
                        nc.tensor.transpose(out=tp[:], in_=gt[:],
                                            identity=id128[:])
                        gT = wpool.tile([DIM, P], f32, tag="gT")
                        nc.vector.tensor_copy(out=gT[:], in_=tp[:])
                        zp = psm.tile([P, DIM], f32, tag="sm")
                        nc.tensor.matmul(out=zp[:], lhsT=gT[:], rhs=Ws[2][:],
                                         start=True, stop=True)
                        zz = wpool.tile([P, DIM], f32, tag="zz")
                        nc.vector.tensor_tensor(out=zz[:], in0=zp[:],
                                                in1=bB[2][:],
                                                op=mybir.AluOpType.add)
                        nc.vector.tensor_scalar(
                            out=zz[:], in0=zz[:], scalar1=0.0, scalar2=None,
                            op0=mybir.AluOpType.max)
                        tp2 = psm.tile([DIM, P], f32, tag="sm")
                        nc.tensor.transpose(out=tp2[:], in_=zz[:],
                                            identity=id128[:])
                        zT = wpool.tile([DIM, P], f32, tag="zT")
                        nc.vector.tensor_copy(out=zT[:], in_=tp2[:])
                        pp = psm.tile([P, 1], f32, tag="sm")
                        nc.tensor.matmul(out=pp[:], lhsT=zT[:], rhs=Ws[3][:],
                                         start=True, stop=True)
                        pr = wpool.tile([P, 1], f32, tag="pr")
                        nc.vector.tensor_tensor(out=pr[:], in0=pp[:],
                                                in1=bB[3][:],
                                                op=mybir.AluOpType.add)
                        nc.sync.dma_start(out=pred_p[qh * P:qh * P + gq, :],
                                          in_=pr[:gq, :])
    nc.compile()
    return nc


# --------------------------------------------------------------- runner ---

def _make_in_maps(x, W1, b1, W2, b2, W3, b3, W4, b4, cfg, per_core):
    c = cfg
    iotaM = np.tile(np.arange(P, dtype=np.float32)[None, :], (P, 1))
    id128 = np.eye(P, dtype=np.float32)
    ones1 = np.ones((1, P), dtype=np.float32)
    maps = []
    for r in range(c.R):
        n0 = r * c.PC
        xs = np.zeros((c.PC, DIM), dtype=np.float32)
        nreal = max(0, min(c.N - n0, c.PC))
        if nreal:
            xs[:nreal] = np.asarray(x[n0:n0 + nreal], dtype=np.float32)
        pc = per_core[r]
        maps.append({
            "xT": np.ascontiguousarray(xs.T),
            "W1": np.asarray(W1, np.float32),
            "W2": np.asarray(W2, np.float32),
            "W3": np.asarray(W3, np.float32),
            "W4": np.asarray(W4, np.float32).reshape(DIM, 1),
            "b1": np.asarray(b1, np.float32).reshape(1, DIM),
            "b2": np.asarray(b2, np.float32).reshape(1, DIM),
            "b3": np.asarray(b3, np.float32).reshape(1, DIM),
            "b4": np.asarray(b4, np.float32).reshape(1, 1),
            "iotaM": iotaM, "id128": id128, "ones1": ones1,
            "idxw": pc["idxw"], "dstl": pc["dstl"], "batchF": pc["batchF"],
            "dinvF": pc["dinvF"], "gciF": pc["gciF"],
            "g0f": np.array([[float(pc["g0"])]], dtype=np.float32),
            "g0i": np.array([[pc["g0"]]], dtype=np.uint32),
        })
    return maps


def kernel(x, edge_index, batch, W1, b1, W2, b2, W3, b3, W4, b4,
           cfg=None, run=None):
    import sys
    if "/opt/trn_rl_repo" not in sys.path:
        sys.path.insert(0, "/opt/trn_rl_repo")
    cfg = cfg or FULL
    x = np.asarray(x)
    edge_index = np.asarray(edge_index)
    batch = np.asarray(batch)
    sched, per_core = _prep(edge_index, batch, cfg)
    nc = build_program(cfg, sched)
    maps = _make_in_maps(x, W1, b1, W2, b2, W3, b3, W4, b4, cfg, per_core)
    if run is not None:                 # custom runner (e.g. simulator)
        return run(nc, maps)
    from concourse.bass_utils import run_bass_kernel_spmd
    res = run_bass_kernel_spmd(nc, maps, list(range(cfg.R)))
    return np.asarray(res.results[0]["pred"]).reshape(-1).astype(np.float32)



# revision 12
# speedup vs baseline: 2.3836x; 2.3836x over previous
"""Trainium2 Bass kernel v5: 2-layer GCN + global mean pool + MLP head.

v4 structure for layer 1 (single whole-table AllGather, per-(bank,group)
dma_gather calls, one-hot aggregation matmuls), plus:
 - layer 2 + mean-pool FOLDED into host-precomputed pooling weights:
   pool_g = sum_m A[g,m] * hs2_m with A[g,m] = sum_{e: src=m} dinv_dst
   + dinv_m*[batch(m)=g].  The per-node layer-2 output is never
   materialized, eliminating the second AllGather and the entire
   second gather/one-hot/matmul aggregation loop (~half the kernel).
 - AllGather output table in Shared scratchpad (fast path per NRT).
"""

import numpy as np

P = 128
DIM = 64
G512 = 512


class CFG:
    def __init__(self, n=100000, e=1600000, g=512, cores=8, maxch_call=10):
        self.N = n
        self.E = e
        self.G = g
        self.R = cores
        self.PC = -(-n // cores)
        self.PC = -(-self.PC // P) * P           # 12544
        self.T = self.PC // P                    # 98
        self.NP = self.PC * cores                # 100352
        self.NROWS = self.NP // 2                # pair-packed table rows
        self.BANK = 32768
        self.NB = -(-self.NROWS // self.BANK)    # 2
        self.MAXCH = maxch_call
        self.nqueues = 4
        self.msg_bufs = 32
        self.skip_gather = False
        self.skip_oh = False
        self.skip_ag = False
        self.skip_tail = False
        self.skip_mm = False
        self.skip_pool = False
        self.debugout = False
        self.repeat = 1


FULL = CFG()


# ---------------------------------------------------- host preprocessing ---

def _prep(edge_index, batch, cfg):
    """v1-style bucketing for the layer-1 aggregation, host-computed dinv,
    per-graph inverse counts, and the fused layer2+pool weight tiles."""
    c = cfg
    src = np.asarray(edge_index[0], dtype=np.int64)
    dst = np.asarray(edge_index[1], dtype=np.int64)
    batch = np.asarray(batch, dtype=np.int64)

    core = dst // c.PC
    tloc = (dst % c.PC) // P
    slot = dst % P
    row2 = src // 2                              # pair-packed table row
    bank = row2 // c.BANK
    par = (src % 2).astype(np.int64)
    ib = (row2 % c.BANK).astype(np.int64)

    key = (((core * c.T + tloc) * c.NB + bank) * 2 + par)
    order = np.lexsort((ib, key))
    key_s = key[order]
    ib_s = ib[order].astype(np.int16)
    slot_s = slot[order].astype(np.float32)

    nkey = c.R * c.T * c.NB * 2
    cnts = np.bincount(key_s, minlength=nkey)
    counts = cnts.reshape(c.R, c.T, c.NB, 2)
    starts_flat = np.concatenate([[0], np.cumsum(cnts)])

    C_tbp = -(-counts.max(axis=0) // P)         # [T, NB, 2]
    for t in range(c.T):
        if C_tbp[t].sum() == 0:
            C_tbp[t, 0, 0] = 1

    groups = []
    t0 = 0
    while t0 < c.T:
        t1 = t0
        while t1 < c.T:
            nch = C_tbp[t0:t1 + 1].sum(axis=(0, 2)).max()
            if nch > c.MAXCH and t1 > t0:
                break
            t1 += 1
        groups.append((t0, t1))
        t0 = t1

    chunk_of = np.zeros((c.T, c.NB, 2), dtype=np.int64)
    calls = []       # (bank, t0, t1, chunk0, nch, idx_col0)
    CH = 0
    icol = 0
    for (t0, t1) in groups:
        for b in range(c.NB):
            ch0 = CH
            for t in range(t0, t1):
                for pp in range(2):
                    chunk_of[t, b, pp] = CH
                    CH += int(C_tbp[t, b, pp])
            nch = CH - ch0
            if nch:
                calls.append((b, t0, t1, ch0, int(nch), icol))
                icol += nch * P // 16
    sched = dict(C_tbp=C_tbp, chunk_of=chunk_of, groups=groups, calls=calls,
                 CH=int(CH), ICOLS=int(icol))

    # host degree -> dinv (self-loop included)
    deg = np.bincount(dst, minlength=c.N).astype(np.float32) + 1.0
    dinv_all = (1.0 / np.sqrt(deg)).astype(np.float32)

    # host per-graph inverse counts for mean pooling
    gcnt = np.maximum(np.bincount(batch, minlength=c.G), 1).astype(np.float32)
    NGT = -(-c.G // P)
    gpad = np.ones(NGT * P, np.float32)
    gpad[:c.G] = gcnt
    gciF = (1.0 / gpad).reshape(NGT, P).T.copy()          # [128, NGT]

    bdst = batch[dst]                            # graph of each edge's dst
    wdst = dinv_all[dst]

    per_core = []
    for r in range(c.R):
        idxw = np.zeros((P, icol), dtype=np.int16)
        dstl = np.full((P, CH), -1.0, dtype=np.float32)
        for (b, t0, t1, ch0, nch, col0) in calls:
            li = np.zeros(nch * P, dtype=np.int16)
            for t in range(t0, t1):
                for pp in range(2):
                    k = ((r * c.T + t) * c.NB + b) * 2 + pp
                    s0, s1 = starts_flat[k], starts_flat[k + 1]
                    n = int(s1 - s0)
                    if n == 0:
                        continue
                    o = int(chunk_of[t, b, pp] - ch0) * P
                    li[o:o + n] = ib_s[s0:s1]
                    cpos = int(chunk_of[t, b, pp])
                    ii = np.arange(n)
                    dstl[ii % P, cpos + ii // P] = slot_s[s0:s1]
            w = li.reshape(-1, 16).T                      # [16, ncol]
            idxw[:, col0:col0 + nch * P // 16] = np.tile(w, (8, 1))
        n0 = r * c.PC
        nreal = max(0, min(c.N - n0, c.PC))
        dvi = np.ones(c.PC, dtype=np.float32)
        if nreal > 0:
            dvi[:nreal] = dinv_all[n0:n0 + nreal]
        dinvF = dvi.reshape(c.T, P).T.copy()              # [128, T]

        # fused layer2+pool weights A[g, m] for local m (see module doc)
        emask = (src >= n0) & (src < n0 + c.PC)
        sl = src[emask] - n0
        aw = np.bincount(sl * G512 + bdst[emask], weights=wdst[emask],
                         minlength=c.PC * G512).astype(np.float32)
        A = aw.reshape(c.PC, G512)
        if nreal > 0:
            lm = np.arange(nreal)
            A[lm, batch[n0:n0 + nreal]] += dinv_all[n0:n0 + nreal]
        # lhsT tile layout: [128 node-slots, T*512 (tile-major graphs)]
        import ml_dtypes
        WgtF = np.ascontiguousarray(
            A.reshape(c.T, P, G512).transpose(1, 0, 2).reshape(P, c.T * G512)
        ).astype(ml_dtypes.bfloat16)
        per_core.append(dict(idxw=idxw, dstl=dstl, dinvF=dinvF, gciF=gciF,
                             WgtF=WgtF))
    return sched, per_core


# ------------------------------------------------------- program builder ---

def build_program(cfg, sched):
    import concourse.bass as bass
    import concourse.bacc as bacc
    import concourse.mybir as mybir
    import concourse.tile as tile
    from concourse.tile import add_dep_helper

    c = cfg
    dt = mybir.dt
    f32 = dt.float32
    bf16 = dt.bfloat16
    ROWE = 2 * DIM                           # 256B pair-packed bf16 rows
    C_tbp, chunk_of, calls = sched["C_tbp"], sched["chunk_of"], sched["calls"]
    CH, ICOLS = sched["CH"], sched["ICOLS"]
    T, NB = c.T, c.NB

    def tile_chunks(t):
        return [(b, pp, k) for b in range(NB) for pp in range(2)
                for k in range(int(C_tbp[t, b, pp]))]

    def call_chunk0(t, b):
        for (bb, tt0, tt1, c0, nn, _c) in calls:
            if bb == b and tt0 <= t < tt1:
                return c0
        raise AssertionError((t, b))

    nc = bacc.Bacc("TRN2", target_bir_lowering=False, debug=False,
                   num_devices=c.R, num_swdge_queues=c.nqueues)

    # ---- I/O ----
    xT_p = nc.declare_dram_parameter("xT", [DIM, c.PC], f32, isOutput=False)
    W_p = [nc.declare_dram_parameter(f"W{i+1}", [DIM, DIM if i < 3 else 1],
                                     f32, isOutput=False) for i in range(4)]
    b_p = [nc.declare_dram_parameter(f"b{i+1}", [1, DIM if i < 3 else 1],
                                     f32, isOutput=False) for i in range(4)]
    iotaM_p = nc.declare_dram_parameter("iotaM", [P, P], f32, isOutput=False)
    id128_p = nc.declare_dram_parameter("id128", [P, P], f32, isOutput=False)
    ones1_p = nc.declare_dram_parameter("ones1", [1, P], f32, isOutput=False)
    idxw_p = nc.declare_dram_parameter("idxw", [P, ICOLS], dt.int16,
                                       isOutput=False)
    dstl_p = nc.declare_dram_parameter("dstl", [P, CH], f32, isOutput=False)
    dinvF_p = nc.declare_dram_parameter("dinvF", [P, T], f32, isOutput=False)
    gciF_p = nc.declare_dram_parameter("gciF", [P, -(-c.G // P)], f32,
                                       isOutput=False)
    Wgt_p = nc.declare_dram_parameter("Wgt", [P, T * G512], bf16,
                                      isOutput=False)
    pred_p = nc.declare_dram_parameter("pred", [c.G, 1], f32, isOutput=True)
    if c.debugout:
        z1dbg_p = nc.declare_dram_parameter("z1dbg", [c.PC, DIM], f32,
                                            isOutput=True)
        hs2dbg_p = nc.declare_dram_parameter("hs2dbg", [c.PC, DIM], f32,
                                             isOutput=True)
        pooldbg_p = nc.declare_dram_parameter("pooldbg", [c.G, DIM], f32,
                                              isOutput=True)

    # ---- internal DRAM (fresh per repeat for clean timing isolation) ----
    hsl_a, tabs_a, pool_loc_a, pool_red_a = [], [], [], []
    for r_ in range(c.repeat):
        hsl_a.append(nc.dram_tensor(f"hs1_local_{r_}", [c.PC, DIM], bf16))
        tabs_a.append(nc.dram_tensor(f"hs1_table_{r_}", [c.NROWS, ROWE],
                                     bf16))
        pool_loc_a.append(nc.dram_tensor(f"pool_local_{r_}", [c.G, DIM], f32))
        pool_red_a.append(nc.dram_tensor(f"pool_red_{r_}", [c.G, DIM],
                                         f32, addr_space="Shared"))

    rg = [list(range(c.R))]

    with tile.TileContext(nc) as tc:
        with (
            tc.tile_pool(name="const", bufs=1) as cpool,
            tc.tile_pool(name="msg", bufs=c.msg_bufs) as mpool,
            tc.tile_pool(name="oh", bufs=6) as ohpool,
            tc.tile_pool(name="work", bufs=3) as wpool,
            tc.tile_pool(name="hs1s", bufs=1) as hs1pool,
            tc.tile_pool(name="hs2t", bufs=4) as hs2pool,
            tc.tile_pool(name="wgt", bufs=8) as wgpool,
            tc.tile_pool(name="ps_agg", bufs=3, space="PSUM") as pagg,
            tc.tile_pool(name="ps_sm", bufs=3, space="PSUM") as psm,
            tc.tile_pool(name="ps_pool", bufs=1, space="PSUM") as ppool,
        ):

            def load(pool, ap, shape, dtype=f32, name=None):
                t = pool.tile(shape, dtype, tag=name)
                nc.sync.dma_start(out=t[:], in_=ap)
                return t

            iotaM = load(cpool, iotaM_p[:, :], [P, P], name="iotaM")
            id128 = load(cpool, id128_p[:, :], [P, P], name="id128")
            ones1 = load(cpool, ones1_p[:, :], [1, P], name="ones1")
            Ws = [load(cpool, W_p[i][:, :], [DIM, DIM if i < 3 else 1],
                       name=f"W{i}") for i in range(4)]
            bs = [load(cpool, b_p[i][:, :], [1, DIM if i < 3 else 1],
                       name=f"b{i}") for i in range(4)]
            idxw = load(cpool, idxw_p[:, :], [P, ICOLS], dt.int16, name="idxw")
            dstl = load(cpool, dstl_p[:, :], [P, CH], name="dstl")
            dinvF = load(cpool, dinvF_p[:, :], [P, T], name="dinvF")
            gciF = load(cpool, gciF_p[:, :], [P, -(-c.G // P)], name="gciF")

            iotaE = cpool.tile([P, P], bf16, tag="iotaE")
            nc.vector.tensor_copy(out=iotaE[:], in_=iotaM[:])

            # bias broadcast mats via PE outer product: ones1.T @ b
            bB = []
            for i in range(4):
                w = DIM if i < 3 else 1
                pb = psm.tile([P, w], f32, tag="sm")
                nc.tensor.matmul(out=pb[:], lhsT=ones1[:], rhs=bs[i][:],
                                 start=True, stop=True)
                sb = cpool.tile([P, w], f32, tag=f"bB{i}")
                nc.vector.tensor_copy(out=sb[:], in_=pb[:])
                bB.append(sb)
            # b1 as a column [DIM,1] for the ACT fused bias
            pb1 = psm.tile([DIM, 1], f32, tag="sm")
            nc.tensor.transpose(out=pb1[:], in_=bs[0][:],
                                identity=id128[0:1, 0:1])
            b1T = cpool.tile([DIM, 1], f32, tag="b1T")
            nc.vector.tensor_copy(out=b1T[:], in_=pb1[:])

            oh_const = cpool.tile([P, P], bf16, tag="ohc")
            nc.vector.memset(oh_const[:], 0.0)

            def onehot_for(chunk):
                if c.skip_oh:
                    return oh_const
                oh = ohpool.tile([P, P], bf16, tag="oh")
                nc.vector.tensor_scalar(
                    out=oh[:], in0=iotaE[:],
                    scalar1=dstl[:, chunk:chunk + 1], scalar2=None,
                    op0=mybir.AluOpType.is_equal)
                return oh

            for _rep in range(c.repeat):
                hsl = hsl_a[_rep]
                tabs = tabs_a[_rep]
                pool_loc = pool_loc_a[_rep]
                pool_red = pool_red_a[_rep]

                hsS = hs1pool.tile([P, T * DIM], bf16, tag="hs1S",
                                   name="hs1S")
                ag_h = [None]
                hs_dmas = []

                def emit_hs(t, hp):
                    """hs = hp * dinv[t] -> bf16 SBUF resident == AG input."""
                    nc.vector.tensor_scalar(
                        out=hsS[:, t * DIM:(t + 1) * DIM], in0=hp[:],
                        scalar1=dinvF[:, t:t + 1], scalar2=None,
                        op0=mybir.AluOpType.mult)
                    d = nc.sync.dma_start(
                        out=hsl[t * P:(t + 1) * P, :],
                        in_=hsS[:, t * DIM:(t + 1) * DIM])
                    hs_dmas.append(d)
                    if t == T - 1 and not c.skip_ag:
                        ag = nc.gpsimd.collective_compute(
                            "AllGather", mybir.AluOpType.bypass,
                            replica_groups=rg,
                            ins=[hsl[:, :]], outs=[tabs[:, :]])
                        for dd in hs_dmas:
                            add_dep_helper(ag.ins, dd.ins)
                        ag_h[0] = ag

                # ---------- layer-1 features: h1 = x @ W1 ----------
                TH_ = T // 2
                with tc.tile_pool(name="xTp", bufs=1) as xpool:
                    for hf in range(2):
                        xh = xpool.tile([DIM, TH_ * P], f32, tag="xh",
                                        name="xh")
                        nc.sync.dma_start(
                            out=xh[:],
                            in_=xT_p[:, hf * TH_ * P:(hf + 1) * TH_ * P])
                        for tt in range(TH_):
                            t = hf * TH_ + tt
                            hp = psm.tile([P, DIM], f32, tag="sm")
                            nc.tensor.matmul(
                                out=hp[:], lhsT=xh[:, tt * P:(tt + 1) * P],
                                rhs=Ws[0][:], start=True, stop=True)
                            emit_hs(t, hp)

                # ---------- layer-1 edge aggregation ----------
                gat_of = {}
                for ci, (b, t0, t1, ch0, nch, col0) in enumerate(calls):
                    m = mpool.tile([P, nch, ROWE], bf16, tag="msg")
                    if not c.skip_gather:
                        g = nc.gpsimd.dma_gather(
                            m[:],
                            tabs[b * c.BANK:
                                 min((b + 1) * c.BANK, c.NROWS), :],
                            idxw[:, col0:col0 + nch * P // 16],
                            nch * P, nch * P, ROWE,
                            single_packet=(nch * P <= 1024),
                            queue_num=ci % c.nqueues)
                        if ag_h[0] is not None:
                            add_dep_helper(g.ins, ag_h[0].ins)
                    else:
                        nc.vector.memset(m[:, 0, :], 0.0)
                    for t in range(t0, t1):
                        gat_of[(t, b)] = m

                psum_pool = ppool.tile([P, 4 * DIM], f32, tag="pool")

                def l1fin(t, pt):
                    # z1 = dinv*(agg + hs1_self); relu(z1^T + b1); @W2
                    zt = wpool.tile([P, DIM], f32, tag="zt")
                    nc.vector.tensor_tensor(
                        out=zt[:], in0=pt[:],
                        in1=hsS[:, t * DIM:(t + 1) * DIM],
                        op=mybir.AluOpType.add)
                    nc.vector.tensor_scalar(
                        out=zt[:], in0=zt[:], scalar1=dinvF[:, t:t + 1],
                        scalar2=None, op0=mybir.AluOpType.mult)
                    tp = psm.tile([DIM, P], f32, tag="sm")
                    nc.tensor.transpose(out=tp[:], in_=zt[:],
                                        identity=id128[:])
                    o1T = wpool.tile([DIM, P], f32, tag="o1T")
                    nc.scalar.activation(
                        out=o1T[:], in_=tp[:],
                        func=mybir.ActivationFunctionType.Relu,
                        bias=b1T[:, 0:1], scale=1.0)
                    hp2 = psm.tile([P, DIM], f32, tag="sm")
                    nc.tensor.matmul(out=hp2[:], lhsT=o1T[:], rhs=Ws[1][:],
                                     start=True, stop=True)
                    # hs2 = h2 * dinv -> bf16, then fused layer2+pool:
                    # pool[g,:] += Wgt[m,g]^T hs2[m,:]  (4 graph blocks)
                    hs2t = hs2pool.tile([P, DIM], bf16, tag="hs2t")
                    nc.vector.tensor_scalar(
                        out=hs2t[:], in0=hp2[:], scalar1=dinvF[:, t:t + 1],
                        scalar2=None, op0=mybir.AluOpType.mult)
                    if c.debugout:
                        nc.sync.dma_start(
                            out=z1dbg_p[t * P:(t + 1) * P, :], in_=zt[:])
                        nc.sync.dma_start(
                            out=hs2dbg_p[t * P:(t + 1) * P, :], in_=hs2t[:])
                    if not c.skip_pool:
                        wg = wgpool.tile([P, G512], bf16, tag="wg")
                        nc.sync.dma_start(
                            out=wg[:],
                            in_=Wgt_p[:, t * G512:(t + 1) * G512])
                        # start=True clears has_written for the WHOLE bank,
                        # so only the first matmul of the bank may set it;
                        # later first-writes overwrite via unset has_written.
                        for k in range(4):
                            nc.tensor.matmul(
                                out=psum_pool[:, k * DIM:(k + 1) * DIM],
                                lhsT=wg[:, k * P:(k + 1) * P],
                                rhs=hs2t[:], start=(t == 0 and k == 0),
                                stop=(t == T - 1))

                for t in range(T):
                    chunks = tile_chunks(t)
                    if c.skip_mm:
                        chunks = chunks[:1]
                    pt = pagg.tile([P, DIM], f32, tag="agg")
                    for j, (b, pp, k) in enumerate(chunks):
                        oh = onehot_for(int(chunk_of[t, b, pp]) + k)
                        m = gat_of[(t, b)]
                        kk = (int(chunk_of[t, b, pp])
                              - call_chunk0(t, b) + k)
                        nc.tensor.matmul(
                            out=pt[:], lhsT=oh[:],
                            rhs=m[:, kk, pp * DIM:(pp + 1) * DIM],
                            start=(j == 0), stop=(j == len(chunks) - 1))
                    l1fin(t, pt)

                # ---------- pool finalize + AllReduce + head ----------
                poolsb = wpool.tile([P, 4 * DIM], f32, tag="poolsb")
                nc.vector.tensor_copy(out=poolsb[:], in_=psum_pool[:])
                if c.debugout:
                    for k in range(4):
                        nc.sync.dma_start(
                            out=pooldbg_p[k * P:(k + 1) * P, :],
                            in_=poolsb[:, k * DIM:(k + 1) * DIM])
                if c.skip_tail:
                    nc.sync.dma_start(out=pred_p[0:P, :], in_=poolsb[:, 0:1])
                else:
                    pdmas = []
                    for k in range(4):
                        pdmas.append(nc.sync.dma_start(
                            out=pool_loc[k * P:(k + 1) * P, :],
                            in_=poolsb[:, k * DIM:(k + 1) * DIM]))
                    ar = nc.gpsimd.collective_compute(
                        "AllReduce", mybir.AluOpType.add, replica_groups=rg,
                        ins=[pool_loc[:, :]], outs=[pool_red[:, :]])
                    for pd in pdmas:
                        add_dep_helper(ar.ins, pd.ins)

                    for qh in range(-(-c.G // P)):
                        gq = min(P, c.G - qh * P)
                        S = wpool.tile([P, DIM], f32, tag="S")
                        d = nc.sync.dma_start(
                            out=S[:], in_=pool_red[qh * P:qh * P + P, :])
                        add_dep_helper(d.ins, ar.ins)
                        gt = wpool.tile([P, DIM], f32, tag="gt")
                        nc.vector.tensor_scalar(
                            out=gt[:], in0=S[:, 0:DIM],
                            scalar1=gciF[:, qh:qh + 1],
                            scalar2=None, op0=mybir.AluOpType.mult)
                        nc.vector.tensor_tensor(out=gt[:], in0=gt[:],
                                                in1=bB[1][:],
                                                op=mybir.AluOpType.add)
                        tp = psm.tile([DIM, P], f32, tag="sm")
                        nc.tensor.transpose(out=tp[:], in_=gt[:],
                                            identity=id128[:])
                        gT = wpool.tile([DIM, P], f32, tag="gT")
                        nc.vector.tensor_copy(out=gT[:], in_=tp[:])
                        zp = psm.tile([P, DIM], f32, tag="sm")
                        nc.tensor.matmul(out=zp[:], lhsT=gT[:], rhs=Ws[2][:],
                                         start=True, stop=True)
                        zz = wpool.tile([P, DIM], f32, tag="zz")
                        nc.vector.tensor_tensor(out=zz[:], in0=zp[:],
                                                in1=bB[2][:],
                                                op=mybir.AluOpType.add)
                        nc.vector.tensor_scalar(
                            out=zz[:], in0=zz[:], scalar1=0.0, scalar2=None,
                            op0=mybir.AluOpType.max)
                        tp2 = psm.tile([DIM, P], f32, tag="sm")
                        nc.tensor.transpose(out=tp2[:], in_=zz[:],
                                            identity=id128[:])
                        zT = wpool.tile([DIM, P], f32, tag="zT")
                        nc.vector.tensor_copy(out=zT[:], in_=tp2[:])
                        pp = psm.tile([P, 1], f32, tag="sm")
                        nc.tensor.matmul(out=pp[:], lhsT=zT[:], rhs=Ws[3][:],
                                         start=True, stop=True)
                        pr = wpool.tile([P, 1], f32, tag="pr")
                        nc.vector.tensor_tensor(out=pr[:], in0=pp[:],
                                                in1=bB[3][:],
                                                op=mybir.AluOpType.add)
                        nc.sync.dma_start(out=pred_p[qh * P:qh * P + gq, :],
                                          in_=pr[:gq, :])
    nc.compile()
    return nc


# --------------------------------------------------------------- runner ---

def _make_in_maps(x, W1, b1, W2, b2, W3, b3, W4, b4, cfg, per_core):
    c = cfg
    iotaM = np.tile(np.arange(P, dtype=np.float32)[None, :], (P, 1))
    id128 = np.eye(P, dtype=np.float32)
    ones1 = np.ones((1, P), dtype=np.float32)
    maps = []
    for r in range(c.R):
        n0 = r * c.PC
        xs = np.zeros((c.PC, DIM), dtype=np.float32)
        nreal = max(0, min(c.N - n0, c.PC))
        if nreal:
            xs[:nreal] = np.asarray(x[n0:n0 + nreal], dtype=np.float32)
        pc = per_core[r]
        maps.append({
            "xT": np.ascontiguousarray(xs.T),
            "W1": np.asarray(W1, np.float32),
            "W2": np.asarray(W2, np.float32),
            "W3": np.asarray(W3, np.float32),
            "W4": np.asarray(W4, np.float32).reshape(DIM, 1),
            "b1": np.asarray(b1, np.float32).reshape(1, DIM),
            "b2": np.asarray(b2, np.float32).reshape(1, DIM),
            "b3": np.asarray(b3, np.float32).reshape(1, DIM),
            "b4": np.asarray(b4, np.float32).reshape(1, 1),
            "iotaM": iotaM, "id128": id128, "ones1": ones1,
            "idxw": pc["idxw"], "dstl": pc["dstl"],
            "dinvF": pc["dinvF"], "gciF": pc["gciF"],
            "Wgt": pc["WgtF"],
        })
    return maps


def kernel(x, edge_index, batch, W1, b1, W2, b2, W3, b3, W4, b4,
           cfg=None, run=None):
    import sys
    if "/opt/trn_rl_repo" not in sys.path:
        sys.path.insert(0, "/opt/trn_rl_repo")
    cfg = cfg or FULL
    x = np.asarray(x)
    edge_index = np.asarray(edge_index)
    batch = np.asarray(batch)
    sched, per_core = _prep(edge_index, batch, cfg)
    nc = build_program(cfg, sched)
    maps = _make_in_maps(x, W1, b1, W2, b2, W3, b3, W4, b4, cfg, per_core)
    if run is not None:                 # custom runner (e.g. simulator)
        return run(nc, maps)
    from concourse.bass_utils import run_bass_kernel_spmd
    res = run_bass_kernel_spmd(nc, maps, list(range(cfg.R)))
    return np.asarray(res.results[0]["pred"]).reshape(-1).astype(np.float32)


# revision 18
# speedup vs baseline: 2.4199x; 1.0152x over previous
"""Trainium2 Bass kernel v5: 2-layer GCN + global mean pool + MLP head.

v4 structure for layer 1 (single whole-table AllGather, per-(bank,group)
dma_gather calls, one-hot aggregation matmuls), plus:
 - layer 2 + mean-pool FOLDED into host-precomputed pooling weights:
   pool_g = sum_m A[g,m] * hs2_m with A[g,m] = sum_{e: src=m} dinv_dst
   + dinv_m*[batch(m)=g].  The per-node layer-2 output is never
   materialized, eliminating the second AllGather and the entire
   second gather/one-hot/matmul aggregation loop (~half the kernel).
 - AllGather output table in Shared scratchpad (fast path per NRT).
"""

import numpy as np

P = 128
DIM = 64
G512 = 512


class CFG:
    def __init__(self, n=100000, e=1600000, g=512, cores=8, maxch_call=20):
        self.N = n
        self.E = e
        self.G = g
        self.R = cores
        self.PC = -(-n // cores)
        self.PC = -(-self.PC // P) * P           # 12544
        self.T = self.PC // P                    # 98
        self.NP = self.PC * cores                # 100352
        self.NROWS = self.NP // 2                # pair-packed table rows
        self.BANK = 32768
        self.NB = -(-self.NROWS // self.BANK)    # 2
        self.MAXCH = maxch_call
        self.nqueues = 4
        self.msg_bufs = 16
        self.skip_gather = False
        self.skip_oh = False
        self.skip_ag = False
        self.skip_tail = False
        self.skip_mm = False
        self.skip_pool = False
        self.debugout = False
        self.repeat = 1


FULL = CFG()


# ---------------------------------------------------- host preprocessing ---

def _prep(edge_index, batch, cfg):
    """v1-style bucketing for the layer-1 aggregation, host-computed dinv,
    per-graph inverse counts, and the fused layer2+pool weight tiles."""
    c = cfg
    src = np.asarray(edge_index[0], dtype=np.int64)
    dst = np.asarray(edge_index[1], dtype=np.int64)
    batch = np.asarray(batch, dtype=np.int64)

    core = dst // c.PC
    tloc = (dst % c.PC) // P
    slot = dst % P
    row2 = src // 2                              # pair-packed table row
    bank = row2 // c.BANK
    par = (src % 2).astype(np.int64)
    ib = (row2 % c.BANK).astype(np.int64)

    key = (((core * c.T + tloc) * c.NB + bank) * 2 + par)
    order = np.lexsort((ib, key))
    key_s = key[order]
    ib_s = ib[order].astype(np.int16)
    slot_s = slot[order].astype(np.float32)

    nkey = c.R * c.T * c.NB * 2
    cnts = np.bincount(key_s, minlength=nkey)
    counts = cnts.reshape(c.R, c.T, c.NB, 2)
    starts_flat = np.concatenate([[0], np.cumsum(cnts)])

    C_tbp = -(-counts.max(axis=0) // P)         # [T, NB, 2]
    for t in range(c.T):
        if C_tbp[t].sum() == 0:
            C_tbp[t, 0, 0] = 1

    groups = []
    t0 = 0
    while t0 < c.T:
        t1 = t0
        while t1 < c.T:
            nch = C_tbp[t0:t1 + 1].sum(axis=(0, 2)).max()
            if nch > c.MAXCH and t1 > t0:
                break
            t1 += 1
        groups.append((t0, t1))
        t0 = t1

    chunk_of = np.zeros((c.T, c.NB, 2), dtype=np.int64)
    calls = []       # (bank, t0, t1, chunk0, nch, idx_col0)
    CH = 0
    icol = 0
    for (t0, t1) in groups:
        for b in range(c.NB):
            ch0 = CH
            for t in range(t0, t1):
                for pp in range(2):
                    chunk_of[t, b, pp] = CH
                    CH += int(C_tbp[t, b, pp])
            nch = CH - ch0
            if nch:
                calls.append((b, t0, t1, ch0, int(nch), icol))
                icol += nch * P // 16
    sched = dict(C_tbp=C_tbp, chunk_of=chunk_of, groups=groups, calls=calls,
                 CH=int(CH), ICOLS=int(icol))

    # host degree -> dinv (self-loop included)
    deg = np.bincount(dst, minlength=c.N).astype(np.float32) + 1.0
    dinv_all = (1.0 / np.sqrt(deg)).astype(np.float32)

    # host per-graph inverse counts for mean pooling (per-core slice)
    gcnt = np.maximum(np.bincount(batch, minlength=c.G), 1).astype(np.float32)
    GS_ = c.G // c.R

    bdst = batch[dst]                            # graph of each edge's dst
    wdst = dinv_all[dst]

    per_core = []
    for r in range(c.R):
        idxw = np.zeros((P, icol), dtype=np.int16)
        dstl = np.full((P, CH), -1.0, dtype=np.float32)   # cast bf16 in maps
        for (b, t0, t1, ch0, nch, col0) in calls:
            li = np.zeros(nch * P, dtype=np.int16)
            for t in range(t0, t1):
                for pp in range(2):
                    k = ((r * c.T + t) * c.NB + b) * 2 + pp
                    s0, s1 = starts_flat[k], starts_flat[k + 1]
                    n = int(s1 - s0)
                    if n == 0:
                        continue
                    o = int(chunk_of[t, b, pp] - ch0) * P
                    li[o:o + n] = ib_s[s0:s1]
                    cpos = int(chunk_of[t, b, pp])
                    ii = np.arange(n)
                    dstl[ii % P, cpos + ii // P] = slot_s[s0:s1]
            w = li.reshape(-1, 16).T                      # [16, ncol]
            idxw[:, col0:col0 + nch * P // 16] = np.tile(w, (8, 1))
        n0 = r * c.PC
        nreal = max(0, min(c.N - n0, c.PC))
        dvi = np.ones(c.PC, dtype=np.float32)
        if nreal > 0:
            dvi[:nreal] = dinv_all[n0:n0 + nreal]
        dinvF = dvi.reshape(c.T, P).T.copy()              # [128, T]

        # fused layer2+pool weights A[g, m] for local m (see module doc)
        emask = (src >= n0) & (src < n0 + c.PC)
        sl = src[emask] - n0
        aw = np.bincount(sl * G512 + bdst[emask], weights=wdst[emask],
                         minlength=c.PC * G512).astype(np.float32)
        A = aw.reshape(c.PC, G512)
        if nreal > 0:
            lm = np.arange(nreal)
            A[lm, batch[n0:n0 + nreal]] += dinv_all[n0:n0 + nreal]
        # lhsT tile layout: [128 node-slots, T*512 (tile-major graphs)]
        import ml_dtypes
        WgtF = np.ascontiguousarray(
            A.reshape(c.T, P, G512).transpose(1, 0, 2).reshape(P, c.T * G512)
        ).astype(ml_dtypes.bfloat16)
        gciS = np.ones((P, 1), np.float32)
        gciS[:GS_, 0] = 1.0 / gcnt[r * GS_:(r + 1) * GS_]
        per_core.append(dict(idxw=idxw, dstl=dstl, dinvF=dinvF, gciS=gciS,
                             WgtF=WgtF))
    return sched, per_core


# ------------------------------------------------------- program builder ---

def build_program(cfg, sched):
    import concourse.bass as bass
    import concourse.bacc as bacc
    import concourse.mybir as mybir
    import concourse.tile as tile
    from concourse.tile import add_dep_helper

    c = cfg
    dt = mybir.dt
    f32 = dt.float32
    bf16 = dt.bfloat16
    ROWE = 2 * DIM                           # 256B pair-packed bf16 rows
    C_tbp, chunk_of, calls = sched["C_tbp"], sched["chunk_of"], sched["calls"]
    CH, ICOLS = sched["CH"], sched["ICOLS"]
    T, NB = c.T, c.NB

    def tile_chunks(t):
        return [(b, pp, k) for b in range(NB) for pp in range(2)
                for k in range(int(C_tbp[t, b, pp]))]

    def call_chunk0(t, b):
        for (bb, tt0, tt1, c0, nn, _c) in calls:
            if bb == b and tt0 <= t < tt1:
                return c0
        raise AssertionError((t, b))

    nc = bacc.Bacc("TRN2", target_bir_lowering=False, debug=False,
                   num_devices=c.R, num_swdge_queues=c.nqueues)

    # ---- I/O ----
    xT_p = nc.declare_dram_parameter("xT", [DIM, c.PC], f32, isOutput=False)
    W_p = [nc.declare_dram_parameter(f"W{i+1}", [DIM, DIM if i < 3 else 1],
                                     f32, isOutput=False) for i in range(4)]
    b_p = [nc.declare_dram_parameter(f"b{i+1}", [1, DIM if i < 3 else 1],
                                     f32, isOutput=False) for i in range(4)]
    iotaM_p = nc.declare_dram_parameter("iotaM", [P, P], f32, isOutput=False)
    id128_p = nc.declare_dram_parameter("id128", [P, P], f32, isOutput=False)
    ones1_p = nc.declare_dram_parameter("ones1", [1, P], f32, isOutput=False)
    idxw_p = nc.declare_dram_parameter("idxw", [P, ICOLS], dt.int16,
                                       isOutput=False)
    dstl_p = nc.declare_dram_parameter("dstl", [P, CH], f32, isOutput=False)
    dinvF_p = nc.declare_dram_parameter("dinvF", [P, T], f32, isOutput=False)
    gciS_p = nc.declare_dram_parameter("gciS", [P, 1], f32, isOutput=False)
    Wgt_p = nc.declare_dram_parameter("Wgt", [P, T * G512], bf16,
                                      isOutput=False)
    GS = c.G // c.R                          # per-core graph slice
    pred_p = nc.declare_dram_parameter("pred", [GS, 1], f32, isOutput=True)
    if c.debugout:
        z1dbg_p = nc.declare_dram_parameter("z1dbg", [c.PC, DIM], f32,
                                            isOutput=True)
        hs2dbg_p = nc.declare_dram_parameter("hs2dbg", [c.PC, DIM], f32,
                                             isOutput=True)
        pooldbg_p = nc.declare_dram_parameter("pooldbg", [c.G, DIM], f32,
                                              isOutput=True)

    # ---- internal DRAM (fresh per repeat for clean timing isolation) ----
    hsl_a, tabs_a, pool_loc_a, pool_red_a = [], [], [], []
    for r_ in range(c.repeat):
        hsl_a.append(nc.dram_tensor(f"hs1_local_{r_}", [c.PC, DIM], bf16))
        tabs_a.append(nc.dram_tensor(f"hs1_table_{r_}", [c.NROWS, ROWE],
                                     bf16))
        pool_loc_a.append(nc.dram_tensor(f"pool_local_{r_}", [c.G, DIM], f32))
        pool_red_a.append(nc.dram_tensor(f"pool_red_{r_}", [c.G // c.R, DIM],
                                         f32))

    rg = [list(range(c.R))]

    with tile.TileContext(nc) as tc:
        with (
            tc.tile_pool(name="const", bufs=1) as cpool,
            tc.tile_pool(name="msg", bufs=c.msg_bufs) as mpool,
            tc.tile_pool(name="oh", bufs=6) as ohpool,
            tc.tile_pool(name="work", bufs=3) as wpool,
            tc.tile_pool(name="hs1s", bufs=1) as hs1pool,
            tc.tile_pool(name="hs2t", bufs=4) as hs2pool,
            tc.tile_pool(name="wgt", bufs=8) as wgpool,
            tc.tile_pool(name="ps_agg", bufs=3, space="PSUM") as pagg,
            tc.tile_pool(name="ps_sm", bufs=3, space="PSUM") as psm,
            tc.tile_pool(name="ps_pool", bufs=1, space="PSUM") as ppool,
        ):

            def load(pool, ap, shape, dtype=f32, name=None):
                t = pool.tile(shape, dtype, tag=name)
                nc.sync.dma_start(out=t[:], in_=ap)
                return t

            iotaM = load(cpool, iotaM_p[:, :], [P, P], name="iotaM")
            id128 = load(cpool, id128_p[:, :], [P, P], name="id128")
            ones1 = load(cpool, ones1_p[:, :], [1, P], name="ones1")
            Ws = [load(cpool, W_p[i][:, :], [DIM, DIM if i < 3 else 1],
                       name=f"W{i}") for i in range(4)]
            bs = [load(cpool, b_p[i][:, :], [1, DIM if i < 3 else 1],
                       name=f"b{i}") for i in range(4)]
            idxw = load(cpool, idxw_p[:, :], [P, ICOLS], dt.int16, name="idxw")
            dstl = load(cpool, dstl_p[:, :], [P, CH], name="dstl")
            dinvF = load(cpool, dinvF_p[:, :], [P, T], name="dinvF")
            gciS = load(cpool, gciS_p[:, :], [P, 1], name="gciS")

            iotaE = cpool.tile([P, P], bf16, tag="iotaE")
            nc.vector.tensor_copy(out=iotaE[:], in_=iotaM[:])

            # bias broadcast mats via PE outer product: ones1.T @ b
            bB = []
            for i in range(4):
                w = DIM if i < 3 else 1
                pb = psm.tile([P, w], f32, tag="sm")
                nc.tensor.matmul(out=pb[:], lhsT=ones1[:], rhs=bs[i][:],
                                 start=True, stop=True)
                sb = cpool.tile([P, w], f32, tag=f"bB{i}")
                nc.vector.tensor_copy(out=sb[:], in_=pb[:])
                bB.append(sb)
            # b1 as a column [DIM,1] for the ACT fused bias
            pb1 = psm.tile([DIM, 1], f32, tag="sm")
            nc.tensor.transpose(out=pb1[:], in_=bs[0][:],
                                identity=id128[0:1, 0:1])
            b1T = cpool.tile([DIM, 1], f32, tag="b1T")
            nc.vector.tensor_copy(out=b1T[:], in_=pb1[:])

            oh_const = cpool.tile([P, P], bf16, tag="ohc")
            nc.vector.memset(oh_const[:], 0.0)

            def onehot_for(chunk):
                if c.skip_oh:
                    return oh_const
                oh = ohpool.tile([P, P], bf16, tag="oh")
                nc.vector.tensor_scalar(
                    out=oh[:], in0=iotaE[:],
                    scalar1=dstl[:, chunk:chunk + 1], scalar2=None,
                    op0=mybir.AluOpType.is_equal)
                return oh

            for _rep in range(c.repeat):
                hsl = hsl_a[_rep]
                tabs = tabs_a[_rep]
                pool_loc = pool_loc_a[_rep]
                pool_red = pool_red_a[_rep]

                hsS = hs1pool.tile([P, T * DIM], bf16, tag="hs1S",
                                   name="hs1S")
                ag_h = [None]
                hs_dmas = []

                def emit_hs(t, hp):
                    """hs = hp * dinv[t] -> bf16 SBUF resident == AG input."""
                    nc.vector.tensor_scalar(
                        out=hsS[:, t * DIM:(t + 1) * DIM], in0=hp[:],
                        scalar1=dinvF[:, t:t + 1], scalar2=None,
                        op0=mybir.AluOpType.mult)
                    d = nc.sync.dma_start(
                        out=hsl[t * P:(t + 1) * P, :],
                        in_=hsS[:, t * DIM:(t + 1) * DIM])
                    hs_dmas.append(d)
                    if t == T - 1 and not c.skip_ag:
                        ag = nc.gpsimd.collective_compute(
                            "AllGather", mybir.AluOpType.bypass,
                            replica_groups=rg,
                            ins=[hsl[:, :]], outs=[tabs[:, :]])
                        for dd in hs_dmas:
                            add_dep_helper(ag.ins, dd.ins)
                        ag_h[0] = ag

                # ---------- layer-1 features: h1 = x @ W1 ----------
                TH_ = T // 2
                with tc.tile_pool(name="xTp", bufs=1) as xpool:
                    for hf in range(2):
                        xh = xpool.tile([DIM, TH_ * P], f32, tag="xh",
                                        name="xh")
                        nc.sync.dma_start(
                            out=xh[:],
                            in_=xT_p[:, hf * TH_ * P:(hf + 1) * TH_ * P])
                        for tt in range(TH_):
                            t = hf * TH_ + tt
                            hp = psm.tile([P, DIM], f32, tag="sm")
                            nc.tensor.matmul(
                                out=hp[:], lhsT=xh[:, tt * P:(tt + 1) * P],
                                rhs=Ws[0][:], start=True, stop=True)
                            emit_hs(t, hp)

                # ---------- layer-1 edge aggregation ----------
                gat_of = {}
                for ci, (b, t0, t1, ch0, nch, col0) in enumerate(calls):
                    m = mpool.tile([P, nch, ROWE], bf16, tag="msg")
                    if not c.skip_gather:
                        g = nc.gpsimd.dma_gather(
                            m[:],
                            tabs[b * c.BANK:
                                 min((b + 1) * c.BANK, c.NROWS), :],
                            idxw[:, col0:col0 + nch * P // 16],
                            nch * P, nch * P, ROWE,
                            single_packet=(nch * P <= 1024),
                            queue_num=ci % c.nqueues)
                        if ag_h[0] is not None:
                            add_dep_helper(g.ins, ag_h[0].ins)
                    else:
                        nc.vector.memset(m[:, 0, :], 0.0)
                    for t in range(t0, t1):
                        gat_of[(t, b)] = m

                psum_pool = ppool.tile([P, 4 * DIM], f32, tag="pool")

                def l1fin(t, pt):
                    # z1 = dinv*(agg + hs1_self); relu(z1^T + b1); @W2
                    zt = wpool.tile([P, DIM], f32, tag="zt")
                    nc.vector.tensor_tensor(
                        out=zt[:], in0=pt[:],
                        in1=hsS[:, t * DIM:(t + 1) * DIM],
                        op=mybir.AluOpType.add)
                    nc.vector.tensor_scalar(
                        out=zt[:], in0=zt[:], scalar1=dinvF[:, t:t + 1],
                        scalar2=None, op0=mybir.AluOpType.mult)
                    tp = psm.tile([DIM, P], f32, tag="sm")
                    nc.tensor.transpose(out=tp[:], in_=zt[:],
                                        identity=id128[:])
                    o1T = wpool.tile([DIM, P], f32, tag="o1T")
                    nc.scalar.activation(
                        out=o1T[:], in_=tp[:],
                        func=mybir.ActivationFunctionType.Relu,
                        bias=b1T[:, 0:1], scale=1.0)
                    hp2 = psm.tile([P, DIM], f32, tag="sm")
                    nc.tensor.matmul(out=hp2[:], lhsT=o1T[:], rhs=Ws[1][:],
                                     start=True, stop=True)
                    # hs2 = h2 * dinv -> bf16, then fused layer2+pool:
                    # pool[g,:] += Wgt[m,g]^T hs2[m,:]  (4 graph blocks)
                    hs2t = hs2pool.tile([P, DIM], bf16, tag="hs2t")
                    nc.vector.tensor_scalar(
                        out=hs2t[:], in0=hp2[:], scalar1=dinvF[:, t:t + 1],
                        scalar2=None, op0=mybir.AluOpType.mult)
                    if c.debugout:
                        nc.sync.dma_start(
                            out=z1dbg_p[t * P:(t + 1) * P, :], in_=zt[:])
                        nc.sync.dma_start(
                            out=hs2dbg_p[t * P:(t + 1) * P, :], in_=hs2t[:])
                    if not c.skip_pool:
                        wg = wgpool.tile([P, G512], bf16, tag="wg")
                        nc.sync.dma_start(
                            out=wg[:],
                            in_=Wgt_p[:, t * G512:(t + 1) * G512])
                        # start=True clears has_written for the WHOLE bank,
                        # so only the first matmul of the bank may set it;
                        # later first-writes overwrite via unset has_written.
                        for k in range(4):
                            nc.tensor.matmul(
                                out=psum_pool[:, k * DIM:(k + 1) * DIM],
                                lhsT=wg[:, k * P:(k + 1) * P],
                                rhs=hs2t[:], start=(t == 0 and k == 0),
                                stop=(t == T - 1))

                for t in range(T):
                    chunks = tile_chunks(t)
                    if c.skip_mm:
                        chunks = chunks[:1]
                    pt = pagg.tile([P, DIM], f32, tag="agg")
                    for j, (b, pp, k) in enumerate(chunks):
                        oh = onehot_for(int(chunk_of[t, b, pp]) + k)
                        m = gat_of[(t, b)]
                        kk = (int(chunk_of[t, b, pp])
                              - call_chunk0(t, b) + k)
                        nc.tensor.matmul(
                            out=pt[:], lhsT=oh[:],
                            rhs=m[:, kk, pp * DIM:(pp + 1) * DIM],
                            start=(j == 0), stop=(j == len(chunks) - 1))
                    l1fin(t, pt)

                # ---------- pool finalize + AllReduce + head ----------
                poolsb = wpool.tile([P, 4 * DIM], f32, tag="poolsb")
                nc.vector.tensor_copy(out=poolsb[:], in_=psum_pool[:])
                if c.debugout:
                    for k in range(4):
                        nc.sync.dma_start(
                            out=pooldbg_p[k * P:(k + 1) * P, :],
                            in_=poolsb[:, k * DIM:(k + 1) * DIM])
                if c.skip_tail:
                    nc.sync.dma_start(out=pred_p[0:GS, :],
                                      in_=poolsb[:GS, 0:1])
                else:
                    pdmas = []
                    for k in range(4):
                        pdmas.append(nc.sync.dma_start(
                            out=pool_loc[k * P:(k + 1) * P, :],
                            in_=poolsb[:, k * DIM:(k + 1) * DIM]))
                    ar = nc.gpsimd.collective_compute(
                        "ReduceScatter", mybir.AluOpType.add,
                        replica_groups=rg,
                        ins=[pool_loc[:, :]], outs=[pool_red[:, :]])
                    for pd in pdmas:
                        add_dep_helper(ar.ins, pd.ins)

                    # head on this core's 64-graph slice only
                    S = wpool.tile([P, DIM], f32, tag="S")
                    nc.vector.memset(S[:], 0.0)
                    d = nc.sync.dma_start(out=S[:GS, :], in_=pool_red[:, :])
                    add_dep_helper(d.ins, ar.ins)
                    gt = wpool.tile([P, DIM], f32, tag="gt")
                    nc.vector.tensor_scalar(
                        out=gt[:], in0=S[:, 0:DIM],
                        scalar1=gciS[:, 0:1],
                        scalar2=None, op0=mybir.AluOpType.mult)
                    nc.vector.tensor_tensor(out=gt[:], in0=gt[:],
                                            in1=bB[1][:],
                                            op=mybir.AluOpType.add)
                    tp = psm.tile([DIM, P], f32, tag="sm")
                    nc.tensor.transpose(out=tp[:], in_=gt[:],
                                        identity=id128[:])
                    gT = wpool.tile([DIM, P], f32, tag="gT")
                    nc.vector.tensor_copy(out=gT[:], in_=tp[:])
                    zp = psm.tile([P, DIM], f32, tag="sm")
                    nc.tensor.matmul(out=zp[:], lhsT=gT[:], rhs=Ws[2][:],
                                     start=True, stop=True)
                    zz = wpool.tile([P, DIM], f32, tag="zz")
                    nc.vector.tensor_tensor(out=zz[:], in0=zp[:],
                                            in1=bB[2][:],
                                            op=mybir.AluOpType.add)
                    nc.vector.tensor_scalar(
                        out=zz[:], in0=zz[:], scalar1=0.0, scalar2=None,
                        op0=mybir.AluOpType.max)
                    tp2 = psm.tile([DIM, P], f32, tag="sm")
                    nc.tensor.transpose(out=tp2[:], in_=zz[:],
                                        identity=id128[:])
                    zT = wpool.tile([DIM, P], f32, tag="zT")
                    nc.vector.tensor_copy(out=zT[:], in_=tp2[:])
                    pp = psm.tile([P, 1], f32, tag="sm")
                    nc.tensor.matmul(out=pp[:], lhsT=zT[:], rhs=Ws[3][:],
                                     start=True, stop=True)
                    pr = wpool.tile([P, 1], f32, tag="pr")
                    nc.vector.tensor_tensor(out=pr[:], in0=pp[:],
                                            in1=bB[3][:],
                                            op=mybir.AluOpType.add)
                    nc.sync.dma_start(out=pred_p[:, :], in_=pr[:GS, :])
    nc.compile()
    return nc


# --------------------------------------------------------------- runner ---

def _make_in_maps(x, W1, b1, W2, b2, W3, b3, W4, b4, cfg, per_core):
    c = cfg
    iotaM = np.tile(np.arange(P, dtype=np.float32)[None, :], (P, 1))
    id128 = np.eye(P, dtype=np.float32)
    ones1 = np.ones((1, P), dtype=np.float32)
    maps = []
    for r in range(c.R):
        n0 = r * c.PC
        xs = np.zeros((c.PC, DIM), dtype=np.float32)
        nreal = max(0, min(c.N - n0, c.PC))
        if nreal:
            xs[:nreal] = np.asarray(x[n0:n0 + nreal], dtype=np.float32)
        pc = per_core[r]
        maps.append({
            "xT": np.ascontiguousarray(xs.T),
            "W1": np.asarray(W1, np.float32),
            "W2": np.asarray(W2, np.float32),
            "W3": np.asarray(W3, np.float32),
            "W4": np.asarray(W4, np.float32).reshape(DIM, 1),
            "b1": np.asarray(b1, np.float32).reshape(1, DIM),
            "b2": np.asarray(b2, np.float32).reshape(1, DIM),
            "b3": np.asarray(b3, np.float32).reshape(1, DIM),
            "b4": np.asarray(b4, np.float32).reshape(1, 1),
            "iotaM": iotaM, "id128": id128, "ones1": ones1,
            "idxw": pc["idxw"], "dstl": pc["dstl"],
            "dinvF": pc["dinvF"], "gciS": pc["gciS"],
            "Wgt": pc["WgtF"],
        })
    return maps


def kernel(x, edge_index, batch, W1, b1, W2, b2, W3, b3, W4, b4,
           cfg=None, run=None):
    import sys
    if "/opt/trn_rl_repo" not in sys.path:
        sys.path.insert(0, "/opt/trn_rl_repo")
    cfg = cfg or FULL
    x = np.asarray(x)
    edge_index = np.asarray(edge_index)
    batch = np.asarray(batch)
    sched, per_core = _prep(edge_index, batch, cfg)
    nc = build_program(cfg, sched)
    maps = _make_in_maps(x, W1, b1, W2, b2, W3, b3, W4, b4, cfg, per_core)
    if run is not None:                 # custom runner (e.g. simulator)
        return run(nc, maps)
    from concourse.bass_utils import run_bass_kernel_spmd
    res = run_bass_kernel_spmd(nc, maps, list(range(cfg.R)))
    return np.concatenate(
        [np.asarray(res.results[r]["pred"]).reshape(-1)
         for r in range(cfg.R)]).astype(np.float32)


# revision 21
# speedup vs baseline: 2.5801x; 1.0662x over previous
"""Trainium2 Bass kernel v5: 2-layer GCN + global mean pool + MLP head.

v4 structure for layer 1 (single whole-table AllGather, per-(bank,group)
dma_gather calls, one-hot aggregation matmuls), plus:
 - layer 2 + mean-pool FOLDED into host-precomputed pooling weights:
   pool_g = sum_m A[g,m] * hs2_m with A[g,m] = sum_{e: src=m} dinv_dst
   + dinv_m*[batch(m)=g].  The per-node layer-2 output is never
   materialized, eliminating the second AllGather and the entire
   second gather/one-hot/matmul aggregation loop (~half the kernel).
 - AllGather output table in Shared scratchpad (fast path per NRT).
"""

import numpy as np

P = 128
DIM = 64
G512 = 512


class CFG:
    def __init__(self, n=100000, e=1600000, g=512, cores=8, maxch_call=20):
        self.N = n
        self.E = e
        self.G = g
        self.R = cores
        self.PC = -(-n // cores)
        self.PC = -(-self.PC // P) * P           # 12544
        self.T = self.PC // P                    # 98
        self.NP = self.PC * cores                # 100352
        self.NROWS = self.NP // 2                # pair-packed table rows
        self.BANK = self.NROWS // 2              # 25088: bank == AG half
        self.NB = -(-self.NROWS // self.BANK)    # 2
        self.MAXCH = maxch_call
        self.nqueues = 4
        self.msg_bufs = 20
        self.skip_gather = False
        self.skip_oh = False
        self.skip_ag = False
        self.skip_tail = False
        self.skip_mm = False
        self.skip_pool = False
        self.debugout = False
        self.repeat = 1


FULL = CFG()


# ---------------------------------------------------- host preprocessing ---

def _prep(edge_index, batch, cfg):
    """v1-style bucketing for the layer-1 aggregation, host-computed dinv,
    per-graph inverse counts, and the fused layer2+pool weight tiles."""
    c = cfg
    src = np.asarray(edge_index[0], dtype=np.int64)
    dst = np.asarray(edge_index[1], dtype=np.int64)
    batch = np.asarray(batch, dtype=np.int64)

    core = dst // c.PC
    tloc = (dst % c.PC) // P
    slot = dst % P
    # table layout is half-major (AG chunk), then core-major, pair-packed:
    # node (rc, loc) -> pair-row hf*BANK + rc*(PC//4) + (loc % (PC//2))//2
    HR2 = c.PC // 2                              # nodes per half per core
    s_core = src // c.PC
    s_loc = src % c.PC
    hf = s_loc // HR2
    bank = hf
    par = (s_loc % 2).astype(np.int64)
    ib = (s_core * (HR2 // 2) + (s_loc % HR2) // 2).astype(np.int64)

    key = (((core * c.T + tloc) * c.NB + bank) * 2 + par)
    order = np.lexsort((ib, key))
    key_s = key[order]
    ib_s = ib[order].astype(np.int16)
    slot_s = slot[order].astype(np.float32)

    nkey = c.R * c.T * c.NB * 2
    cnts = np.bincount(key_s, minlength=nkey)
    counts = cnts.reshape(c.R, c.T, c.NB, 2)
    starts_flat = np.concatenate([[0], np.cumsum(cnts)])

    C_tbp = -(-counts.max(axis=0) // P)         # [T, NB, 2]
    for t in range(c.T):
        if C_tbp[t].sum() == 0:
            C_tbp[t, 0, 0] = 1

    groups = []
    t0 = 0
    while t0 < c.T:
        t1 = t0
        while t1 < c.T:
            nch = C_tbp[t0:t1 + 1].sum(axis=(0, 2)).max()
            if nch > c.MAXCH and t1 > t0:
                break
            t1 += 1
        groups.append((t0, t1))
        t0 = t1

    chunk_of = np.zeros((c.T, c.NB, 2), dtype=np.int64)
    calls = []       # (bank, t0, t1, chunk0, nch, idx_col0)
    CH = 0
    icol = 0
    for (t0, t1) in groups:
        for b in range(c.NB):
            ch0 = CH
            for t in range(t0, t1):
                for pp in range(2):
                    chunk_of[t, b, pp] = CH
                    CH += int(C_tbp[t, b, pp])
            nch = CH - ch0
            if nch:
                calls.append((b, t0, t1, ch0, int(nch), icol))
                icol += nch * P // 16
    sched = dict(C_tbp=C_tbp, chunk_of=chunk_of, groups=groups, calls=calls,
                 CH=int(CH), ICOLS=int(icol))

    # host degree -> dinv (self-loop included)
    deg = np.bincount(dst, minlength=c.N).astype(np.float32) + 1.0
    dinv_all = (1.0 / np.sqrt(deg)).astype(np.float32)

    # host per-graph inverse counts for mean pooling (per-core slice)
    gcnt = np.maximum(np.bincount(batch, minlength=c.G), 1).astype(np.float32)
    GS_ = c.G // c.R

    bdst = batch[dst]                            # graph of each edge's dst
    wdst = dinv_all[dst]

    per_core = []
    for r in range(c.R):
        idxw = np.zeros((P, icol), dtype=np.int16)
        dstl = np.full((P, CH), -1.0, dtype=np.float32)   # cast bf16 in maps
        for (b, t0, t1, ch0, nch, col0) in calls:
            li = np.zeros(nch * P, dtype=np.int16)
            for t in range(t0, t1):
                for pp in range(2):
                    k = ((r * c.T + t) * c.NB + b) * 2 + pp
                    s0, s1 = starts_flat[k], starts_flat[k + 1]
                    n = int(s1 - s0)
                    if n == 0:
                        continue
                    o = int(chunk_of[t, b, pp] - ch0) * P
                    li[o:o + n] = ib_s[s0:s1]
                    cpos = int(chunk_of[t, b, pp])
                    ii = np.arange(n)
                    dstl[ii % P, cpos + ii // P] = slot_s[s0:s1]
            w = li.reshape(-1, 16).T                      # [16, ncol]
            idxw[:, col0:col0 + nch * P // 16] = np.tile(w, (8, 1))
        n0 = r * c.PC
        nreal = max(0, min(c.N - n0, c.PC))
        dvi = np.ones(c.PC, dtype=np.float32)
        if nreal > 0:
            dvi[:nreal] = dinv_all[n0:n0 + nreal]
        dinvF = dvi.reshape(c.T, P).T.copy()              # [128, T]

        # fused layer2+pool weights A[g, m] for local m (see module doc)
        emask = (src >= n0) & (src < n0 + c.PC)
        sl = src[emask] - n0
        aw = np.bincount(sl * G512 + bdst[emask], weights=wdst[emask],
                         minlength=c.PC * G512).astype(np.float32)
        A = aw.reshape(c.PC, G512)
        if nreal > 0:
            lm = np.arange(nreal)
            A[lm, batch[n0:n0 + nreal]] += dinv_all[n0:n0 + nreal]
        # lhsT tile layout: [128 node-slots, T*512 (tile-major graphs)]
        import ml_dtypes
        WgtF = np.ascontiguousarray(
            A.reshape(c.T, P, G512).transpose(1, 0, 2).reshape(P, c.T * G512)
        ).astype(ml_dtypes.bfloat16)
        gciS = np.ones((P, 1), np.float32)
        gciS[:GS_, 0] = 1.0 / gcnt[r * GS_:(r + 1) * GS_]
        per_core.append(dict(idxw=idxw, dstl=dstl, dinvF=dinvF, gciS=gciS,
                             WgtF=WgtF))
    return sched, per_core


# ------------------------------------------------------- program builder ---

def build_program(cfg, sched):
    import concourse.bass as bass
    import concourse.bacc as bacc
    import concourse.mybir as mybir
    import concourse.tile as tile
    from concourse.tile import add_dep_helper

    c = cfg
    dt = mybir.dt
    f32 = dt.float32
    bf16 = dt.bfloat16
    ROWE = 2 * DIM                           # 256B pair-packed bf16 rows
    C_tbp, chunk_of, calls = sched["C_tbp"], sched["chunk_of"], sched["calls"]
    CH, ICOLS = sched["CH"], sched["ICOLS"]
    T, NB = c.T, c.NB

    def tile_chunks(t):
        return [(b, pp, k) for b in range(NB) for pp in range(2)
                for k in range(int(C_tbp[t, b, pp]))]

    def call_chunk0(t, b):
        for (bb, tt0, tt1, c0, nn, _c) in calls:
            if bb == b and tt0 <= t < tt1:
                return c0
        raise AssertionError((t, b))

    nc = bacc.Bacc("TRN2", target_bir_lowering=False, debug=False,
                   num_devices=c.R, num_swdge_queues=c.nqueues)

    # ---- I/O ----
    xT_p = nc.declare_dram_parameter("xT", [DIM, c.PC], f32, isOutput=False)
    W_p = [nc.declare_dram_parameter(f"W{i+1}", [DIM, DIM if i < 3 else 1],
                                     f32, isOutput=False) for i in range(4)]
    b_p = [nc.declare_dram_parameter(f"b{i+1}", [1, DIM if i < 3 else 1],
                                     f32, isOutput=False) for i in range(4)]
    iotaM_p = nc.declare_dram_parameter("iotaM", [P, P], f32, isOutput=False)
    id128_p = nc.declare_dram_parameter("id128", [P, P], f32, isOutput=False)
    ones1_p = nc.declare_dram_parameter("ones1", [1, P], f32, isOutput=False)
    idxw_p = nc.declare_dram_parameter("idxw", [P, ICOLS], dt.int16,
                                       isOutput=False)
    dstl_p = nc.declare_dram_parameter("dstl", [P, CH], f32, isOutput=False)
    dinvF_p = nc.declare_dram_parameter("dinvF", [P, T], f32, isOutput=False)
    gciS_p = nc.declare_dram_parameter("gciS", [P, 1], f32, isOutput=False)
    Wgt_p = nc.declare_dram_parameter("Wgt", [P, T * G512], bf16,
                                      isOutput=False)
    GS = c.G // c.R                          # per-core graph slice
    pred_p = nc.declare_dram_parameter("pred", [GS, 1], f32, isOutput=True)
    if c.debugout:
        z1dbg_p = nc.declare_dram_parameter("z1dbg", [c.PC, DIM], f32,
                                            isOutput=True)
        hs2dbg_p = nc.declare_dram_parameter("hs2dbg", [c.PC, DIM], f32,
                                             isOutput=True)
        pooldbg_p = nc.declare_dram_parameter("pooldbg", [c.G, DIM], f32,
                                              isOutput=True)

    # ---- internal DRAM (fresh per repeat for clean timing isolation) ----
    hsl_a, tabs_a, pool_loc_a, pool_red_a = [], [], [], []
    for r_ in range(c.repeat):
        hsl_a.append(nc.dram_tensor(f"hs1_local_{r_}", [c.PC, DIM], bf16))
        tabs_a.append(nc.dram_tensor(f"hs1_table_{r_}", [c.NROWS, ROWE],
                                     bf16))
        pool_loc_a.append(nc.dram_tensor(f"pool_local_{r_}", [c.G, DIM], f32))
        pool_red_a.append(nc.dram_tensor(f"pool_red_{r_}", [c.G // c.R, DIM],
                                         f32))

    rg = [list(range(c.R))]

    with tile.TileContext(nc) as tc:
        with (
            tc.tile_pool(name="const", bufs=1) as cpool,
            tc.tile_pool(name="msg", bufs=c.msg_bufs) as mpool,
            tc.tile_pool(name="oh", bufs=6) as ohpool,
            tc.tile_pool(name="work", bufs=3) as wpool,
            tc.tile_pool(name="hs1s", bufs=1) as hs1pool,
            tc.tile_pool(name="hs2t", bufs=4) as hs2pool,
            tc.tile_pool(name="wgt", bufs=8) as wgpool,
            tc.tile_pool(name="ps_agg", bufs=3, space="PSUM") as pagg,
            tc.tile_pool(name="ps_sm", bufs=3, space="PSUM") as psm,
            tc.tile_pool(name="ps_pool", bufs=1, space="PSUM") as ppool,
        ):

            def load(pool, ap, shape, dtype=f32, name=None):
                t = pool.tile(shape, dtype, tag=name)
                nc.sync.dma_start(out=t[:], in_=ap)
                return t

            iotaM = load(cpool, iotaM_p[:, :], [P, P], name="iotaM")
            id128 = load(cpool, id128_p[:, :], [P, P], name="id128")
            ones1 = load(cpool, ones1_p[:, :], [1, P], name="ones1")
            Ws = [load(cpool, W_p[i][:, :], [DIM, DIM if i < 3 else 1],
                       name=f"W{i}") for i in range(4)]
            bs = [load(cpool, b_p[i][:, :], [1, DIM if i < 3 else 1],
                       name=f"b{i}") for i in range(4)]
            idxw = load(cpool, idxw_p[:, :], [P, ICOLS], dt.int16, name="idxw")
            dstl = load(cpool, dstl_p[:, :], [P, CH], name="dstl")
            dinvF = load(cpool, dinvF_p[:, :], [P, T], name="dinvF")
            gciS = load(cpool, gciS_p[:, :], [P, 1], name="gciS")

            iotaE = cpool.tile([P, P], bf16, tag="iotaE")
            nc.vector.tensor_copy(out=iotaE[:], in_=iotaM[:])

            # bias broadcast mats via PE outer product: ones1.T @ b
            bB = []
            for i in range(4):
                w = DIM if i < 3 else 1
                pb = psm.tile([P, w], f32, tag="sm")
                nc.tensor.matmul(out=pb[:], lhsT=ones1[:], rhs=bs[i][:],
                                 start=True, stop=True)
                sb = cpool.tile([P, w], f32, tag=f"bB{i}")
                nc.vector.tensor_copy(out=sb[:], in_=pb[:])
                bB.append(sb)
            # b1 as a column [DIM,1] for the ACT fused bias
            pb1 = psm.tile([DIM, 1], f32, tag="sm")
            nc.tensor.transpose(out=pb1[:], in_=bs[0][:],
                                identity=id128[0:1, 0:1])
            b1T = cpool.tile([DIM, 1], f32, tag="b1T")
            nc.vector.tensor_copy(out=b1T[:], in_=pb1[:])

            oh_const = cpool.tile([P, P], bf16, tag="ohc")
            nc.vector.memset(oh_const[:], 0.0)

            def onehot_for(chunk):
                if c.skip_oh:
                    return oh_const
                oh = ohpool.tile([P, P], bf16, tag="oh")
                nc.vector.tensor_scalar(
                    out=oh[:], in0=iotaE[:],
                    scalar1=dstl[:, chunk:chunk + 1], scalar2=None,
                    op0=mybir.AluOpType.is_equal)
                return oh

            for _rep in range(c.repeat):
                hsl = hsl_a[_rep]
                tabs = tabs_a[_rep]
                pool_loc = pool_loc_a[_rep]
                pool_red = pool_red_a[_rep]

                hsS = hs1pool.tile([P, T * DIM], bf16, tag="hs1S",
                                   name="hs1S")
                ag_h = [None, None]
                hs_dmas = []
                TH2 = T // 2                     # tiles per AG half (49)
                HR = c.PC // 2                   # local rows per half (6272)

                def emit_hs(t, hp):
                    """hs = hp * dinv[t] -> bf16 SBUF resident == AG input."""
                    nc.vector.tensor_scalar(
                        out=hsS[:, t * DIM:(t + 1) * DIM], in0=hp[:],
                        scalar1=dinvF[:, t:t + 1], scalar2=None,
                        op0=mybir.AluOpType.mult)
                    d = nc.sync.dma_start(
                        out=hsl[t * P:(t + 1) * P, :],
                        in_=hsS[:, t * DIM:(t + 1) * DIM])
                    hs_dmas.append(d)
                    if (t + 1) % TH2 == 0 and not c.skip_ag:
                        hf = t // TH2            # 0 or 1
                        # table half hf: rows [hf*R*HR/2, ...): bank == half
                        ag = nc.gpsimd.collective_compute(
                            "AllGather", mybir.AluOpType.bypass,
                            replica_groups=rg,
                            ins=[hsl[hf * HR:(hf + 1) * HR, :]],
                            outs=[tabs[hf * c.BANK:(hf + 1) * c.BANK, :]])
                        for dd in hs_dmas:
                            add_dep_helper(ag.ins, dd.ins)
                        hs_dmas.clear()
                        ag_h[hf] = ag

                # ---------- layer-1 features: h1 = x @ W1 ----------
                TH_ = 14                         # 7 groups of 14 tiles
                with tc.tile_pool(name="xTp", bufs=2) as xpool:
                    for hf in range(T // TH_):
                        xh = xpool.tile([DIM, TH_ * P], f32, tag="xh",
                                        name="xh")
                        nc.sync.dma_start(
                            out=xh[:],
                            in_=xT_p[:, hf * TH_ * P:(hf + 1) * TH_ * P])
                        for tt in range(TH_):
                            t = hf * TH_ + tt
                            hp = psm.tile([P, DIM], f32, tag="sm")
                            nc.tensor.matmul(
                                out=hp[:], lhsT=xh[:, tt * P:(tt + 1) * P],
                                rhs=Ws[0][:], start=True, stop=True)
                            emit_hs(t, hp)

                # ---------- layer-1 edge aggregation ----------
                gat_of = {}
                for ci, (b, t0, t1, ch0, nch, col0) in enumerate(calls):
                    m = mpool.tile([P, nch, ROWE], bf16, tag="msg")
                    if not c.skip_gather:
                        g = nc.gpsimd.dma_gather(
                            m[:],
                            tabs[b * c.BANK:
                                 min((b + 1) * c.BANK, c.NROWS), :],
                            idxw[:, col0:col0 + nch * P // 16],
                            nch * P, nch * P, ROWE,
                            single_packet=(nch * P <= 1024),
                            queue_num=ci % c.nqueues)
                        if ag_h[b] is not None:
                            add_dep_helper(g.ins, ag_h[b].ins)
                    else:
                        nc.vector.memset(m[:, 0, :], 0.0)
                    for t in range(t0, t1):
                        gat_of[(t, b)] = m

                psum_pool = ppool.tile([P, 4 * DIM], f32, tag="pool")

                def l1fin(t, pt):
                    # z1 = dinv*(agg + hs1_self); relu(z1^T + b1); @W2
                    zt = wpool.tile([P, DIM], f32, tag="zt")
                    nc.vector.tensor_tensor(
                        out=zt[:], in0=pt[:],
                        in1=hsS[:, t * DIM:(t + 1) * DIM],
                        op=mybir.AluOpType.add)
                    nc.vector.tensor_scalar(
                        out=zt[:], in0=zt[:], scalar1=dinvF[:, t:t + 1],
                        scalar2=None, op0=mybir.AluOpType.mult)
                    tp = psm.tile([DIM, P], f32, tag="sm")
                    nc.tensor.transpose(out=tp[:], in_=zt[:],
                                        identity=id128[:])
                    o1T = wpool.tile([DIM, P], f32, tag="o1T")
                    nc.scalar.activation(
                        out=o1T[:], in_=tp[:],
                        func=mybir.ActivationFunctionType.Relu,
                        bias=b1T[:, 0:1], scale=1.0)
                    hp2 = psm.tile([P, DIM], f32, tag="sm")
                    nc.tensor.matmul(out=hp2[:], lhsT=o1T[:], rhs=Ws[1][:],
                                     start=True, stop=True)
                    # hs2 = h2 * dinv -> bf16, then fused layer2+pool:
                    # pool[g,:] += Wgt[m,g]^T hs2[m,:]  (4 graph blocks)
                    hs2t = hs2pool.tile([P, DIM], bf16, tag="hs2t")
                    nc.vector.tensor_scalar(
                        out=hs2t[:], in0=hp2[:], scalar1=dinvF[:, t:t + 1],
                        scalar2=None, op0=mybir.AluOpType.mult)
                    if c.debugout:
                        nc.sync.dma_start(
                            out=z1dbg_p[t * P:(t + 1) * P, :], in_=zt[:])
                        nc.sync.dma_start(
                            out=hs2dbg_p[t * P:(t + 1) * P, :], in_=hs2t[:])
                    if not c.skip_pool:
                        wg = wgpool.tile([P, G512], bf16, tag="wg")
                        nc.sync.dma_start(
                            out=wg[:],
                            in_=Wgt_p[:, t * G512:(t + 1) * G512])
                        # start=True clears has_written for the WHOLE bank,
                        # so only the first matmul of the bank may set it;
                        # later first-writes overwrite via unset has_written.
                        for k in range(4):
                            nc.tensor.matmul(
                                out=psum_pool[:, k * DIM:(k + 1) * DIM],
                                lhsT=wg[:, k * P:(k + 1) * P],
                                rhs=hs2t[:], start=(t == 0 and k == 0),
                                stop=(t == T - 1))

                for t in range(T):
                    chunks = tile_chunks(t)
                    if c.skip_mm:
                        chunks = chunks[:1]
                    pt = pagg.tile([P, DIM], f32, tag="agg")
                    for j, (b, pp, k) in enumerate(chunks):
                        oh = onehot_for(int(chunk_of[t, b, pp]) + k)
                        m = gat_of[(t, b)]
                        kk = (int(chunk_of[t, b, pp])
                              - call_chunk0(t, b) + k)
                        nc.tensor.matmul(
                            out=pt[:], lhsT=oh[:],
                            rhs=m[:, kk, pp * DIM:(pp + 1) * DIM],
                            start=(j == 0), stop=(j == len(chunks) - 1))
                    l1fin(t, pt)

                # ---------- pool finalize + AllReduce + head ----------
                poolsb = wpool.tile([P, 4 * DIM], f32, tag="poolsb")
                nc.vector.tensor_copy(out=poolsb[:], in_=psum_pool[:])
                if c.debugout:
                    for k in range(4):
                        nc.sync.dma_start(
                            out=pooldbg_p[k * P:(k + 1) * P, :],
                            in_=poolsb[:, k * DIM:(k + 1) * DIM])
                if c.skip_tail:
                    nc.sync.dma_start(out=pred_p[0:GS, :],
                                      in_=poolsb[:GS, 0:1])
                else:
                    pdmas = []
                    for k in range(4):
                        pdmas.append(nc.sync.dma_start(
                            out=pool_loc[k * P:(k + 1) * P, :],
                            in_=poolsb[:, k * DIM:(k + 1) * DIM]))
                    ar = nc.gpsimd.collective_compute(
                        "ReduceScatter", mybir.AluOpType.add,
                        replica_groups=rg,
                        ins=[pool_loc[:, :]], outs=[pool_red[:, :]])
                    for pd in pdmas:
                        add_dep_helper(ar.ins, pd.ins)

                    # head on this core's 64-graph slice only
                    S = wpool.tile([P, DIM], f32, tag="S")
                    nc.vector.memset(S[:], 0.0)
                    d = nc.sync.dma_start(out=S[:GS, :], in_=pool_red[:, :])
                    add_dep_helper(d.ins, ar.ins)
                    gt = wpool.tile([P, DIM], f32, tag="gt")
                    nc.vector.tensor_scalar(
                        out=gt[:], in0=S[:, 0:DIM],
                        scalar1=gciS[:, 0:1],
                        scalar2=None, op0=mybir.AluOpType.mult)
                    nc.vector.tensor_tensor(out=gt[:], in0=gt[:],
                                            in1=bB[1][:],
                                            op=mybir.AluOpType.add)
                    tp = psm.tile([DIM, P], f32, tag="sm")
                    nc.tensor.transpose(out=tp[:], in_=gt[:],
                                        identity=id128[:])
                    gT = wpool.tile([DIM, P], f32, tag="gT")
                    nc.vector.tensor_copy(out=gT[:], in_=tp[:])
                    zp = psm.tile([P, DIM], f32, tag="sm")
                    nc.tensor.matmul(out=zp[:], lhsT=gT[:], rhs=Ws[2][:],
                                     start=True, stop=True)
                    zz = wpool.tile([P, DIM], f32, tag="zz")
                    nc.vector.tensor_tensor(out=zz[:], in0=zp[:],
                                            in1=bB[2][:],
                                            op=mybir.AluOpType.add)
                    nc.vector.tensor_scalar(
                        out=zz[:], in0=zz[:], scalar1=0.0, scalar2=None,
                        op0=mybir.AluOpType.max)
                    tp2 = psm.tile([DIM, P], f32, tag="sm")
                    nc.tensor.transpose(out=tp2[:], in_=zz[:],
                                        identity=id128[:])
                    zT = wpool.tile([DIM, P], f32, tag="zT")
                    nc.vector.tensor_copy(out=zT[:], in_=tp2[:])
                    pp = psm.tile([P, 1], f32, tag="sm")
                    nc.tensor.matmul(out=pp[:], lhsT=zT[:], rhs=Ws[3][:],
                                     start=True, stop=True)
                    pr = wpool.tile([P, 1], f32, tag="pr")
                    nc.vector.tensor_tensor(out=pr[:], in0=pp[:],
                                            in1=bB[3][:],
                                            op=mybir.AluOpType.add)
                    nc.sync.dma_start(out=pred_p[:, :], in_=pr[:GS, :])
    nc.compile()
    return nc


# --------------------------------------------------------------- runner ---

def _make_in_maps(x, W1, b1, W2, b2, W3, b3, W4, b4, cfg, per_core):
    c = cfg
    iotaM = np.tile(np.arange(P, dtype=np.float32)[None, :], (P, 1))
    id128 = np.eye(P, dtype=np.float32)
    ones1 = np.ones((1, P), dtype=np.float32)
    maps = []
    for r in range(c.R):
        n0 = r * c.PC
        xs = np.zeros((c.PC, DIM), dtype=np.float32)
        nreal = max(0, min(c.N - n0, c.PC))
        if nreal:
            xs[:nreal] = np.asarray(x[n0:n0 + nreal], dtype=np.float32)
        pc = per_core[r]
        maps.append({
            "xT": np.ascontiguousarray(xs.T),
            "W1": np.asarray(W1, np.float32),
            "W2": np.asarray(W2, np.float32),
            "W3": np.asarray(W3, np.float32),
            "W4": np.asarray(W4, np.float32).reshape(DIM, 1),
            "b1": np.asarray(b1, np.float32).reshape(1, DIM),
            "b2": np.asarray(b2, np.float32).reshape(1, DIM),
            "b3": np.asarray(b3, np.float32).reshape(1, DIM),
            "b4": np.asarray(b4, np.float32).reshape(1, 1),
            "iotaM": iotaM, "id128": id128, "ones1": ones1,
            "idxw": pc["idxw"], "dstl": pc["dstl"],
            "dinvF": pc["dinvF"], "gciS": pc["gciS"],
            "Wgt": pc["WgtF"],
        })
    return maps


def kernel(x, edge_index, batch, W1, b1, W2, b2, W3, b3, W4, b4,
           cfg=None, run=None):
    import sys
    if "/opt/trn_rl_repo" not in sys.path:
        sys.path.insert(0, "/opt/trn_rl_repo")
    cfg = cfg or FULL
    x = np.asarray(x)
    edge_index = np.asarray(edge_index)
    batch = np.asarray(batch)
    sched, per_core = _prep(edge_index, batch, cfg)
    nc = build_program(cfg, sched)
    maps = _make_in_maps(x, W1, b1, W2, b2, W3, b3, W4, b4, cfg, per_core)
    if run is not None:                 # custom runner (e.g. simulator)
        return run(nc, maps)
    from concourse.bass_utils import run_bass_kernel_spmd
    res = run_bass_kernel_spmd(nc, maps, list(range(cfg.R)))
    return np.concatenate(
        [np.asarray(res.results[r]["pred"]).reshape(-1)
         for r in range(cfg.R)]).astype(np.float32)
